# revision 36
# baseline (speedup 1.0000x reference)
"""Trainium2 Bass kernel for nn_Block_78022375899354 (dense transformer block).

Sharding (8 cores): core c -> batch b=c//2, head-half hh=c%2.
  Phase 1 (self-attn): head-split -- each core computes q/k/v for its 8 heads
    over the full batch; causal attention produces the core's 512 output
    channels for ALL tokens; a pairwise AllToAll exchanges channel halves so
    each core ends with all 1024 attn channels for its OWN 512 tokens.
  Phase 2+3 (cross-attn, MLP, adapter): token-split (512 tokens per core).

Precision: fp8e4 (DoubleRow, 2x PE) for qkv / aproj / cross-attn q,k,v /
caproj GEMMs with power-of-2 scales folded into drains (verified 3.2e-3
rel err on CPU sim vs 2e-2 budget); bf16 for scores/av/fc/mproj; f32 residual.
All reciprocals via exp(-ln(x)) on ACT so the whole attention region uses one
activation table set (natural_log_exp) -- no table thrash, no slow DVE recip.
Scores for the head pair (2h, 2h+1) run concurrently in PE row-groups 0-63 /
64-127; their exps share one 2-bank ACT op. LN gain/bias and projection
biases folded host-side.
"""
import sys
sys.path.insert(0, '/opt/trn_rl_repo')
import numpy as np
import ml_dtypes

BF = ml_dtypes.bfloat16
F8 = ml_dtypes.float8_e4m3fn
P = 128
C = 1024
T = 1024
TE = 257
TEP = 384          # padded encoder length (3 chunks of 128)
NCH = C // P       # 8 channel chunks
F = 512            # free-dim tile (tokens per core)
H = 16
D = 64
EPS = 1e-5

SW = 1024.0        # fp8 weight scale
SLN = 16.0         # fp8 scale for layernorm outputs + encoder embd
SATT = 32.0        # fp8 scale for attention outputs
DS_QKV = 1.0 / (SW * SLN)      # 2^-14
DS_ATT = 1.0 / (SW * SATT)     # 2^-15
LN32 = float(np.log(SATT))

_BUILT = {}


def _build_nc():
    import concourse.bass as bass
    import concourse.mybir as mybir
    import concourse.tile as tile
    from contextlib import ExitStack

    f32 = mybir.dt.float32
    f32r = mybir.dt.float32r
    bf16 = mybir.dt.bfloat16
    fp8 = mybir.dt.float8e4
    AF = mybir.ActivationFunctionType
    ALU = mybir.AluOpType
    DR = mybir.MatmulPerfMode.DoubleRow

    nc = bass.Bass("TRN2", num_devices=8)

    # ---------------- DRAM I/O ----------------
    xT = nc.dram_tensor("xT", [C, T], bf16, kind="ExternalInput")
    x_ownT = nc.dram_tensor("x_ownT", [C, F], f32, kind="ExternalInput")
    encT = nc.dram_tensor("encT", [C, TEP], fp8, kind="ExternalInput")
    wqkv = nc.dram_tensor("wqkv", [C, 1536], fp8, kind="ExternalInput")
    bqk = nc.dram_tensor("bqk", [1024], f32, kind="ExternalInput")
    waproj = nc.dram_tensor("waproj", [512, C], fp8, kind="ExternalInput")
    battn = nc.dram_tensor("battn", [C], f32, kind="ExternalInput")
    wca = nc.dram_tensor("wca", [C, 3 * C], fp8, kind="ExternalInput")
    bcaqk = nc.dram_tensor("bcaqk", [2 * C], f32, kind="ExternalInput")
    wcaproj = nc.dram_tensor("wcaproj", [C, C], fp8, kind="ExternalInput")
    bcaproj = nc.dram_tensor("bcaproj", [C], f32, kind="ExternalInput")
    wfc = nc.dram_tensor("wfc", [C, 4 * C], bf16, kind="ExternalInput")
    bfc = nc.dram_tensor("bfc", [4 * C], f32, kind="ExternalInput")
    wmproj = nc.dram_tensor("wmproj", [NCH, P, 4 * C], bf16, kind="ExternalInput")
    bmproj = nc.dram_tensor("bmproj", [C], f32, kind="ExternalInput")
    wdown = nc.dram_tensor("wdown", [C, 256], bf16, kind="ExternalInput")
    bdown = nc.dram_tensor("bdown", [256], f32, kind="ExternalInput")
    wup = nc.dram_tensor("wup", [256, C], bf16, kind="ExternalInput")
    bup = nc.dram_tensor("bup", [C], f32, kind="ExternalInput")
    out_d = nc.dram_tensor("out", [C, F], f32, kind="ExternalOutput")

    def r3(ap):
        return ap.rearrange("(o p) f -> p o f", p=P)

    def r2(ap):
        return ap.rearrange("(o p) -> p o", p=P)

    with tile.TileContext(nc) as tc:
        with ExitStack() as ctx:
            consts = ctx.enter_context(tc.tile_pool(name="consts", bufs=1))
            work = ctx.enter_context(tc.tile_pool(name="work", bufs=2))
            lns = ctx.enter_context(tc.tile_pool(name="lns", bufs=2))
            dram = ctx.enter_context(tc.tile_pool(name="dram", bufs=1, space="DRAM"))
            # PSUM budget: 2x [P,2,F] "st" (4 banks) + 2x [P,2,F] "ps1" (4)
            ps2 = ctx.enter_context(tc.tile_pool(name="ps2", bufs=2, space="PSUM"))
            ps1 = ctx.enter_context(tc.tile_pool(name="ps1", bufs=2, space="PSUM"))
            lnxb_pool = ctx.enter_context(tc.tile_pool(name="lnxb_pool", bufs=1))
            x2pool = ctx.enter_context(tc.tile_pool(name="x2pool", bufs=1))
            pool_wfc = ctx.enter_context(tc.tile_pool(name="pool_wfc", bufs=2))

            # ---------- constants ----------
            ones_col_bf = consts.tile([P, 1], bf16)
            nc.vector.memset(ones_col_bf, 1.0)
            ones_f32_tmp = consts.tile([1, P], f32)
            nc.vector.memset(ones_f32_tmp, 1.0)
            ones_row_f32 = consts.tile([1, P], f32r)
            nc.scalar.copy(ones_row_f32, ones_f32_tmp)
            ones_bc = consts.tile([1, D], f32r)
            nc.scalar.copy(ones_bc, ones_f32_tmp[:, 0:D])
            # encoder pad handling: rows>0 of kt-chunk 2 get exp bias -1e30 -> 0
            padbias = consts.tile([P, 1], f32)
            nc.vector.memset(padbias, -1e30)
            nc.vector.memset(padbias[0:1, :], 0.0)
            eps_s = consts.tile([P, 1], f32)        # eps / SLN^2 (ln arg bias)
            nc.vector.memset(eps_s, EPS / (SLN * SLN))
            eps_p = consts.tile([P, 1], f32)
            nc.vector.memset(eps_p, EPS)
            ln32_t = consts.tile([P, 1], f32)
            nc.vector.memset(ln32_t, LN32)

            # ---------- bias tiles ----------
            def bias_tile(dr_t, ncols):
                t = consts.tile([P, ncols], f32, tag=f"b_{dr_t.name}",
                                name=f"b_{dr_t.name}")
                nc.sync.dma_start(t, r2(dr_t[:]))
                return t
            bqk_sb = bias_tile(bqk, 8)
            battn_sb = bias_tile(battn, NCH)
            bcaqk_sb = bias_tile(bcaqk, 16)
            bcaproj_sb = bias_tile(bcaproj, NCH)
            bfc_sb = bias_tile(bfc, 32)
            bmproj_sb = bias_tile(bmproj, NCH)
            bdown_sb = bias_tile(bdown, 2)
            bup_sb = bias_tile(bup, NCH)

            # pools live only through phases 1-2 (freed before the MLP)
            actx = ExitStack()
            exp_pool = actx.enter_context(tc.tile_pool(name="exp_pool", bufs=3))
            anorm = actx.enter_context(tc.tile_pool(name="anorm", bufs=2))
            wstream = actx.enter_context(tc.tile_pool(name="wstream", bufs=2))
            kvpool = actx.enter_context(tc.tile_pool(name="kvpool", bufs=1))

            # ---------- layernorm (feature-major), rsqrt via exp(-.5 ln) ----
            # xb: bf16 [P, NCH, ntok]; ln_out scaled by S (16 for fp8 outs).
            def layernorm(xb, ntok, ln_out, sq_scale, eps_t):
                stats = []
                for nt in range(ntok // F):
                    sl = slice(nt * F, (nt + 1) * F)
                    s1 = ps1.tile([1, F], f32, tag="ps1")
                    s2 = ps1.tile([1, F], f32, tag="ps1")
                    for kc in range(NCH):
                        nc.tensor.matmul(s1, ones_col_bf, xb[:, kc, sl],
                                         start=(kc == 0), stop=(kc == NCH - 1))
                    for kc in range(NCH):
                        xsq = work.tile([P, F], bf16, tag="lnxsq")
                        nc.vector.tensor_mul(xsq, xb[:, kc, sl], xb[:, kc, sl])
                        nc.tensor.matmul(s2, ones_col_bf, xsq,
                                         start=(kc == 0), stop=(kc == NCH - 1))
                    # copy sums out of PSUM promptly to recycle the slots
                    s1r = lns.tile([1, F], f32r, tag="m")
                    nc.scalar.copy(s1r, s1)
                    s2r = lns.tile([1, F], f32r, tag="v")
                    nc.scalar.copy(s2r, s2)
                    stats.append((sl, s1r, s2r))
                for sl, s1r, s2r in stats:
                    # broadcast raw sums to all partitions (f32r single-cycle)
                    psS0 = ps2.tile([P, F], f32, tag="st")
                    psS1 = ps2.tile([P, F], f32, tag="st")
                    nc.tensor.matmul(psS0, ones_row_f32, s1r,
                                     start=True, stop=True)
                    nc.tensor.matmul(psS1, ones_row_f32, s2r,
                                     start=True, stop=True)
                    mt = work.tile([P, F], f32, tag="lnmt")
                    nc.vector.tensor_scalar_mul(mt, psS0, 1.0 / C)
                    var = work.tile([P, F], f32, tag="lnvar")
                    # var = s2/C - m*m
                    nc.vector.scalar_tensor_tensor(
                        var, in0=mt, scalar=-1.0, in1=mt, op0=ALU.mult,
                        op1=ALU.mult)
                    nc.vector.scalar_tensor_tensor(
                        var, in0=psS1, scalar=1.0 / C, in1=var,
                        op0=ALU.mult, op1=ALU.add)
                    # A = S*rsqrt(var+eps) = exp(-0.5 ln(var/S^2 + eps/S^2))
                    nc.scalar.activation(var, var, AF.Ln, bias=eps_t[:, 0:1],
                                         scale=sq_scale)
                    A_sb = work.tile([P, F], bf16, tag="lnA")
                    nc.scalar.activation(A_sb, var, AF.Exp, scale=-0.5)
                    B_sb = work.tile([P, F], bf16, tag="lnB")
                    nc.vector.scalar_tensor_tensor(
                        B_sb, in0=mt, scalar=-1.0, in1=A_sb,
                        op0=ALU.mult, op1=ALU.mult)
                    for kc in range(NCH):
                        tmp = work.tile([P, F], bf16, tag="lntmp")
                        nc.vector.tensor_mul(tmp, xb[:, kc, sl], A_sb)
                        nc.vector.tensor_add(ln_out[:, kc, sl], tmp, B_sb)

            # attention normalize pair: rb = exp(ln32 - ln(den)) = 32/den,
            # broadcast across partitions via PE, multiply numerators (PSUM).
            def attn_norm_pair(pav, dstA, dstB):
                lnden = anorm.tile([1, 2, F], f32r, tag="lnden")
                nc.scalar.activation(lnden, pav[64:65, :, :], AF.Ln)
                pbc = ps2.tile([D, 2, F], f32, tag="st")
                for h in range(2):
                    nc.tensor.matmul(pbc[:, h, :], ones_bc,
                                     lnden[:, h, :],
                                     start=True, stop=True)
                rb = anorm.tile([D, 2, F], bf16, tag="rbc")
                nc.scalar.activation(rb, pbc, AF.Exp, scale=-1.0,
                                     bias=ln32_t[0:D, 0:1])
                nc.vector.tensor_mul(dstA, pav[0:D, 0, :], rb[:, 0, :])
                nc.vector.tensor_mul(dstB, pav[0:D, 1, :], rb[:, 1, :])

            # two half-RSs: shard j of half qt -> rank j's token quarter.
            # partials quantized to fp8 (scale 64) to halve the wire.
            Q = 256
            SRS = 64.0
            cc_in = [dram.tile([2, C, Q], fp8, tag=f"ccin{qt}",
                               name=f"ccin{qt}") for qt in range(2)]
            cc_out = [dram.tile([C, Q], fp8, tag=f"ccout{qt}",
                                name=f"ccout{qt}") for qt in range(2)]

            # =================================================================
            # Phase 1: self-attention (head-split, full batch)
            # =================================================================
            with ExitStack() as p1:
                pool_ln1 = p1.enter_context(tc.tile_pool(name="pool_ln1", bufs=1))
                ln1T = pool_ln1.tile([P, NCH, T], fp8)
                with tc.tile_pool(name="pool_x", bufs=1) as pool_x:
                    xT_sb = pool_x.tile([P, NCH, T], bf16)
                    nc.sync.dma_start(xT_sb, r3(xT[:]))
                    layernorm(xT_sb, T, ln1T, 1.0 / (SLN * SLN), eps_s)

                pool_p1 = p1.enter_context(tc.tile_pool(name="pool_p1", bufs=1))
                # causal diagonal-block mask (duplicated for the head
                # pair): dmask[i, h, j] = 1 iff j >= i, applied to the 128
                # columns at the diagonal; columns left of it are skipped
                # entirely, columns right of it are always live.
                dmask = pool_p1.tile([P, 2, P], bf16)
                for h in range(2):
                    nc.gpsimd.memset(dmask[:, h, :], 1.0)
                    nc.gpsimd.affine_select(
                        out=dmask[:, h, :], in_=dmask[:, h, :],
                        compare_op=ALU.is_ge, fill=0.0,
                        base=0, channel_multiplier=-1, pattern=[[1, P]],
                    )
                pool_wq = ExitStack()
                pool_wq_p = pool_wq.enter_context(
                    tc.tile_pool(name="pool_wq", bufs=1))
                wqkv_sb = pool_wq_p.tile([P, NCH, 1536], fp8)
                nc.sync.dma_start(wqkv_sb, r3(wqkv[:]))

                q_sb = pool_p1.tile([P, 4, T], bf16)
                k_sb = pool_p1.tile([P, 4, T], bf16)
                for m in range(4):
                    for ntk in range(T // F):
                        sl = slice(ntk * F, (ntk + 1) * F)
                        for dst, woff, boff in ((q_sb, 0, 0), (k_sb, 512, 4)):
                            pt = ps1.tile([P, F], f32, tag="ps1")
                            for kcp in range(0, NCH, 2):
                                nc.tensor.matmul(
                                    pt,
                                    wqkv_sb[:, kcp:kcp + 2,
                                            woff + m * P:woff + (m + 1) * P],
                                    ln1T[:, kcp:kcp + 2, sl],
                                    start=(kcp == 0), stop=(kcp == NCH - 2),
                                    perf_mode=DR)
                            nc.scalar.activation(
                                dst[:, m, sl], pt, AF.Identity,
                                bias=bqk_sb[:, boff + m:boff + m + 1],
                                scale=DS_QKV)

                waproj_sb = pool_p1.tile([P, 4, C], fp8)
                nc.sync.dma_start(waproj_sb, r3(waproj[:]))

                v_sb = pool_p1.tile([P, NCH, 8, 65], bf16)
                nc.vector.memset(v_sb[:, :, :, 64:65], 1.0)
                for tkc in range(NCH):
                    pt = ps1.tile([P, F], f32, tag="ps1")
                    for kcp in range(0, NCH, 2):
                        nc.tensor.matmul(
                            pt, ln1T[:, kcp:kcp + 2, tkc * P:(tkc + 1) * P],
                            wqkv_sb[:, kcp:kcp + 2, 1024:1536],
                            start=(kcp == 0), stop=(kcp == NCH - 2),
                            perf_mode=DR)
                    nc.scalar.activation(
                        v_sb[:, tkc, :, 0:64],
                        pt.rearrange("p (h d) -> p h d", h=8),
                        AF.Identity, scale=DS_QKV)

                pool_wq.close()

                # encoder K/V (independent work: fills PE gaps in the
                # LN/attention stretches and overlaps the collectives)
                encT_sb = kvpool.tile([P, NCH, TEP], fp8)
                nc.sync.dma_start(encT_sb, r3(encT[:]))
                kc_sb = kvpool.tile([P, NCH, TEP], bf16)
                wca_k = wstream.tile([P, NCH, C], fp8, tag="w8k")
                nc.sync.dma_start(wca_k, r3(wca[:, C:2 * C]))
                for m in range(NCH):
                    pt = ps1.tile([P, TEP], f32, tag="ps1")
                    for kcp in range(0, NCH, 2):
                        nc.tensor.matmul(pt,
                                         wca_k[:, kcp:kcp + 2, m * P:(m + 1) * P],
                                         encT_sb[:, kcp:kcp + 2, :],
                                         start=(kcp == 0), stop=(kcp == NCH - 2),
                                         perf_mode=DR)
                    nc.vector.tensor_scalar(kc_sb[:, m, :], pt, DS_QKV,
                                            bcaqk_sb[:, 8 + m:8 + m + 1],
                                            op0=ALU.mult, op1=ALU.add)
                vc_sb = kvpool.tile([P, 3, H, 65], bf16)
                nc.vector.memset(vc_sb[:, :, :, 64:65], 1.0)
                wca_v = wstream.tile([P, NCH, C], fp8, tag="w8k")
                nc.sync.dma_start(wca_v, r3(wca[:, 2 * C:3 * C]))
                for tkc in range(3):
                    for nh in range(2):
                        pt = ps1.tile([P, F], f32, tag="ps1")
                        for kcp in range(0, NCH, 2):
                            nc.tensor.matmul(
                                pt, encT_sb[:, kcp:kcp + 2, tkc * P:(tkc + 1) * P],
                                wca_v[:, kcp:kcp + 2, nh * F:(nh + 1) * F],
                                start=(kcp == 0), stop=(kcp == NCH - 2),
                                perf_mode=DR)
                        nc.vector.tensor_scalar_mul(
                            vc_sb[:, tkc, nh * 8:(nh + 1) * 8, 0:64],
                            pt.rearrange("p (h d) -> p h d", h=8), DS_QKV)

                attn_sb = pool_p1.tile([P, 4, T], fp8)
                pending = None
                for qt in range(2):
                    qsl = slice(qt * F, (qt + 1) * F)
                    nkc = 4 * (qt + 1)
                    for hch in range(4):
                        hA, hB = 2 * hch, 2 * hch + 1
                        pav = ps1.tile([65, 2, F], f32, tag="ps1")
                        for kc in range(nkc):
                            ksl = slice(kc * P, (kc + 1) * P)
                            dk = kc - 4 * qt
                            c0 = max(0, P * dk)   # first live column
                            qs2 = slice(qt * F + c0, (qt + 1) * F)
                            st = ps2.tile([P, 2, F], f32, tag="st")
                            nc.tensor.matmul(st[:, 0, c0:],
                                             k_sb[0:64, hch, ksl],
                                             q_sb[0:64, hch, qs2],
                                             start=True, stop=True)
                            nc.tensor.matmul(st[:, 1, c0:],
                                             k_sb[64:128, hch, ksl],
                                             q_sb[64:128, hch, qs2],
                                             start=True, stop=True)
                            e = exp_pool.tile([P, 2, F], bf16, tag="exp")
                            nc.scalar.activation(e[:, :, c0:], st[:, :, c0:],
                                                 AF.Exp, scale=0.125)
                            if dk >= 0:
                                nc.vector.tensor_mul(e[:, :, c0:c0 + P],
                                                     e[:, :, c0:c0 + P],
                                                     dmask)
                            nc.tensor.matmul(pav[:, 0, c0:], v_sb[:, kc, hA, :],
                                             e[:, 0, c0:], start=(kc == 0),
                                             stop=(kc == nkc - 1))
                            nc.tensor.matmul(pav[:, 1, c0:], v_sb[:, kc, hB, :],
                                             e[:, 1, c0:], start=(kc == 0),
                                             stop=(kc == nkc - 1))
                        if pending is not None:
                            attn_norm_pair(*pending)
                        pending = (pav, attn_sb[0:64, hch, qsl],
                                   attn_sb[64:128, hch, qsl])
                    attn_norm_pair(*pending)
                    pending = None
                    # attn-proj partial over my 512 channels for this half,
                    # then reduce-scatter it while later work proceeds
                    part = anorm.tile([P, NCH, F], fp8, tag="part")
                    for m in range(NCH):
                        pt = ps2.tile([P, F], f32, tag="st")
                        for kcp in range(0, 4, 2):
                            nc.tensor.matmul(
                                pt, waproj_sb[:, kcp:kcp + 2, m * P:(m + 1) * P],
                                attn_sb[:, kcp:kcp + 2, qsl],
                                start=(kcp == 0), stop=(kcp == 2),
                                perf_mode=DR)
                        nc.vector.tensor_scalar_mul(part[:, m, :], pt,
                                                    DS_ATT * SRS)
                    for j in range(2):
                        nc.sync.dma_start(
                            r3(cc_in[qt][j]),
                            part[:, :, j * Q:(j + 1) * Q])
                    nc.gpsimd.collective_compute(
                        "ReduceScatter", ALU.add,
                        replica_groups=[[0, 1], [2, 3], [4, 5], [6, 7]],
                        ins=[cc_in[qt][:]], outs=[cc_out[qt][:]])

            # =================================================================
            # Phase 2: cross-attention (token-split, own 512 tokens)
            # =================================================================
            with ExitStack() as p2:
                pool_p2 = p2.enter_context(tc.tile_pool(name="pool_p2", bufs=1))
                # prefetch first MLP weight quarters (overlaps collective+ph2)
                wfcq = {}
                def wfc_quarter(qtr):
                    t = pool_wfc.tile([P, NCH, C], bf16, tag="wfcq",
                                      name=f"wfcq{qtr}")
                    nc.sync.dma_start(t, r3(wfc[:, qtr * C:(qtr + 1) * C]))
                    wfcq[qtr] = t
                wfc_quarter(0)
                wfc_quarter(1)

                # x_own = x + battn + RS'd attn-proj halves
                x_own = pool_p2.tile([P, NCH, F], f32)
                nc.sync.dma_start(x_own, r3(x_ownT[:]))
                rs_sb = pool_p2.tile([P, 2, NCH, Q], fp8)
                for qt in range(2):
                    nc.sync.dma_start(rs_sb[:, qt], r3(cc_out[qt][:]))
                for m in range(NCH):
                    nc.gpsimd.tensor_scalar_add(x_own[:, m, :], x_own[:, m, :],
                                                battn_sb[:, m:m + 1])
                xb2 = lnxb_pool.tile([P, NCH, F], bf16, tag="lnxb")
                for m in range(NCH):
                    for qt in range(2):
                        osl = slice(qt * Q, (qt + 1) * Q)
                        nc.vector.scalar_tensor_tensor(
                            xb2[:, m, osl], in0=rs_sb[:, qt, m, :],
                            scalar=1.0 / SRS,
                            in1=x_own[:, m, osl], op0=ALU.mult, op1=ALU.add)
                for m in range(NCH):
                    for qt in range(2):
                        osl = slice(qt * Q, (qt + 1) * Q)
                        nc.vector.scalar_tensor_tensor(
                            x_own[:, m, osl], in0=rs_sb[:, qt, m, :],
                            scalar=1.0 / SRS,
                            in1=x_own[:, m, osl], op0=ALU.mult, op1=ALU.add)
                ln2T = pool_p2.tile([P, NCH, F], fp8)
                layernorm(xb2, F, ln2T, 1.0 / (SLN * SLN), eps_s)
                # pre-add caproj bias into the residual (consumed at drain)
                for m in range(NCH):
                    nc.gpsimd.tensor_scalar_add(x_own[:, m, :], x_own[:, m, :],
                                                bcaproj_sb[:, m:m + 1])

                qc_sb = pool_p2.tile([P, NCH, F], bf16)
                wca_q = wstream.tile([P, NCH, C], fp8, tag="w8k")
                nc.sync.dma_start(wca_q, r3(wca[:, 0:C]))
                for m in range(NCH):
                    pt = ps1.tile([P, F], f32, tag="ps1")
                    for kcp in range(0, NCH, 2):
                        nc.tensor.matmul(pt,
                                         wca_q[:, kcp:kcp + 2, m * P:(m + 1) * P],
                                         ln2T[:, kcp:kcp + 2, :],
                                         start=(kcp == 0), stop=(kcp == NCH - 2),
                                         perf_mode=DR)
                    nc.vector.tensor_scalar(qc_sb[:, m, :], pt, DS_QKV,
                                            bcaqk_sb[:, m:m + 1],
                                            op0=ALU.mult, op1=ALU.add)

                attnc_sb = pool_p2.tile([P, NCH, F], fp8)
                pending = None
                for hch in range(NCH):
                    hA, hB = 2 * hch, 2 * hch + 1
                    pav = ps1.tile([65, 2, F], f32, tag="ps1")
                    for kc in range(3):
                        ksl = slice(kc * P, (kc + 1) * P)
                        st = ps2.tile([P, 2, F], f32, tag="st")
                        nc.tensor.matmul(st[:, 0, :], kc_sb[0:64, hch, ksl],
                                         qc_sb[0:64, hch, :],
                                         start=True, stop=True)
                        nc.tensor.matmul(st[:, 1, :], kc_sb[64:128, hch, ksl],
                                         qc_sb[64:128, hch, :],
                                         start=True, stop=True)
                        e = exp_pool.tile([P, 2, F], bf16, tag="exp")
                        if kc == 2:
                            nc.scalar.activation(e, st, AF.Exp, scale=0.125,
                                                 bias=padbias[:, 0:1])
                        else:
                            nc.scalar.activation(e, st, AF.Exp, scale=0.125)
                        nc.tensor.matmul(pav[:, 0, :], vc_sb[:, kc, hA, :],
                                         e[:, 0, :], start=(kc == 0),
                                         stop=(kc == 2))
                        nc.tensor.matmul(pav[:, 1, :], vc_sb[:, kc, hB, :],
                                         e[:, 1, :], start=(kc == 0),
                                         stop=(kc == 2))
                    if pending is not None:
                        attn_norm_pair(*pending)
                    pending = (pav, attnc_sb[0:64, hch, :],
                               attnc_sb[64:128, hch, :])
                attn_norm_pair(*pending)

                x2 = x2pool.tile([P, NCH, F], f32)
                wcaproj_sb = wstream.tile([P, NCH, C], fp8, tag="w8k")
                nc.sync.dma_start(wcaproj_sb, r3(wcaproj[:]))
                for m in range(NCH):
                    pt = ps1.tile([P, F], f32, tag="ps1")
                    for kcp in range(0, NCH, 2):
                        nc.tensor.matmul(
                            pt, wcaproj_sb[:, kcp:kcp + 2, m * P:(m + 1) * P],
                            attnc_sb[:, kcp:kcp + 2, :],
                            start=(kcp == 0), stop=(kcp == NCH - 2),
                            perf_mode=DR)
                    nc.vector.scalar_tensor_tensor(
                        x2[:, m, :], in0=pt, scalar=DS_ATT,
                        in1=x_own[:, m, :], op0=ALU.mult, op1=ALU.add)

            actx.close()

            # =================================================================
            # Phase 3: MLP + adapter (token-split)
            # =================================================================
            with ExitStack() as p3:
                pool_p3 = p3.enter_context(tc.tile_pool(name="pool_p3", bufs=1))
                xb3 = lnxb_pool.tile([P, NCH, F], bf16, tag="lnxb")
                for kc in range(NCH):
                    nc.vector.tensor_copy(xb3[:, kc, :], x2[:, kc, :])
                ln3T = pool_p3.tile([P, NCH, F], bf16)
                layernorm(xb3, F, ln3T, 1.0, eps_p)

                gT = pool_p3.tile([P, 32, F], bf16)
                for qtr in range(4):
                    if qtr + 2 < 4:
                        wfc_quarter(qtr + 2)
                    for m8 in range(8):
                        mq = qtr * 8 + m8
                        pt = ps1.tile([P, F], f32, tag="ps1")
                        for kc in range(NCH):
                            nc.tensor.matmul(
                                pt, wfcq[qtr][:, kc, m8 * P:(m8 + 1) * P],
                                ln3T[:, kc, :],
                                start=(kc == 0), stop=(kc == NCH - 1))
                        nc.scalar.activation(gT[:, mq, :], pt,
                                             AF.Gelu_apprx_tanh,
                                             bias=bfc_sb[:, mq:mq + 1])

                h_sb = pool_p3.tile([P, NCH, F], bf16)
                wmp_pool = p3.enter_context(tc.tile_pool(name="wmp_pool", bufs=2))
                for m in range(NCH):
                    wmp_t = wmp_pool.tile([P, 32, P], bf16, tag="wmp")
                    nc.sync.dma_start(
                        wmp_t, wmproj[m].rearrange("p (o f) -> p o f", f=P))
                    pt = ps1.tile([P, F], f32, tag="ps1")
                    for kc in range(32):
                        nc.tensor.matmul(pt, wmp_t[:, kc, :], gT[:, kc, :],
                                         start=(kc == 0), stop=(kc == 31))
                    nc.vector.tensor_scalar_add(h_sb[:, m, :], pt,
                                                bmproj_sb[:, m:m + 1])

                wdown_sb = pool_p3.tile([P, NCH, 256], bf16)
                nc.sync.dma_start(wdown_sb, r3(wdown[:]))
                wup_sb = pool_p3.tile([P, 2, C], bf16)
                nc.sync.dma_start(wup_sb, r3(wup[:]))

                aT = pool_p3.tile([P, 2, F], bf16)
                for m in range(2):
                    pt = ps1.tile([P, F], f32, tag="ps1")
                    for kc in range(NCH):
                        nc.tensor.matmul(pt, wdown_sb[:, kc, m * P:(m + 1) * P],
                                         h_sb[:, kc, :],
                                         start=(kc == 0), stop=(kc == NCH - 1))
                    nc.scalar.activation(aT[:, m, :], pt, AF.Gelu_apprx_tanh,
                                         bias=bdown_sb[:, m:m + 1])

                for m in range(NCH):
                    pt = ps1.tile([P, F], f32, tag="ps1")
                    for kc in range(2):
                        nc.tensor.matmul(pt, wup_sb[:, kc, m * P:(m + 1) * P],
                                         aT[:, kc, :], start=(kc == 0),
                                         stop=(kc == 1))
                    tmp = pool_p3.tile([P, F], f32, tag="fin", bufs=2)
                    nc.vector.scalar_tensor_tensor(
                        tmp, in0=pt, scalar=bup_sb[:, m:m + 1], in1=h_sb[:, m, :],
                        op0=ALU.add, op1=ALU.add)
                    fin = pool_p3.tile([P, F], f32, tag="fin2", bufs=2)
                    nc.vector.tensor_add(fin, tmp, x2[:, m, :])
                    nc.sync.dma_start(out_d[m * P:(m + 1) * P, :], fin)

    _split_sync_waits(nc, mybir)
    return nc


def _split_sync_waits(nc, mybir, maxw=1):
    # walrus rejects instructions with more than a couple of sync waits
    # (e.g. the Tile epilogue Drain waits on every engine + DMA queue);
    # move excess waits onto preceding same-engine no-ops.
    for f in nc.m.functions:
        for bb in f.blocks:
            out, changed = [], False
            for ins in bb.instructions:
                si = ins.sync_info
                if si is not None and len(si.on_wait) > maxw:
                    waits = list(si.on_wait)
                    k = 0
                    while len(waits) > maxw:
                        chunk, waits = waits[:maxw], waits[maxw:]
                        nop = mybir.InstNoOp(name=f"{ins.name}-w{k}", ins=[], outs=[])
                        nop.engine = ins.engine
                        nop.sync_info = mybir.SyncInfo(on_wait=chunk, on_update=[])
                        out.append(nop)
                        k += 1
                    ins.sync_info = mybir.SyncInfo(
                        on_wait=waits, on_update=list(si.on_update))
                    changed = True
                out.append(ins)
            if changed:
                bb.instructions = out


def _q8(a, scale):
    return np.clip(np.asarray(a, np.float32) * scale, -240.0, 240.0).astype(F8)


def _prep_inputs(inputs):
    f = lambda k: np.asarray(inputs[k], np.float32)
    x = f('x')
    enc = f('encoder_embd')
    ln1_g, ln1_b = f('ln1_g'), f('ln1_b')
    ln2_g, ln2_b = f('ln2_g'), f('ln2_b')
    ln3_g, ln3_b = f('ln3_g'), f('ln3_b')
    attn_w, attn_b = f('attn_w'), f('attn_b')
    aproj_w, aproj_b = f('aproj_w'), f('aproj_b')
    ca_w, ca_b = f('ca_w'), f('ca_b')
    caproj_w, caproj_b = f('caproj_w'), f('caproj_b')
    fc_w, fc_b = f('fc_w'), f('fc_b')
    mproj_w, mproj_b = f('mproj_w'), f('mproj_b')
    down_w, down_b = f('down_w'), f('down_b')
    up_w, up_b = f('up_w'), f('up_b')

    # fold LN affine into consuming weights (exact for g=1,b=0 fills)
    aw = ln1_g[:, None] * attn_w
    ab = ln1_b @ attn_w + attn_b
    caw_q = ln2_g[:, None] * ca_w[:, :C]
    cab_q = ln2_b @ ca_w[:, :C] + ca_b[:C]
    fw = ln3_g[:, None] * fc_w
    fb = ln3_b @ fc_w + fc_b

    battn = aproj_b + ab[2 * C:] @ aproj_w            # v-bias folded
    bcaproj = caproj_b + ca_b[2 * C:] @ caproj_w

    wca_full = np.concatenate([caw_q, ca_w[:, C:2 * C], ca_w[:, 2 * C:]], 1)
    bcaqk = np.concatenate([cab_q, ca_b[C:2 * C]]).astype(np.float32)

    shared = dict(
        wca=_q8(wca_full, SW), bcaqk=bcaqk,
        battn=battn.astype(np.float32),
        wcaproj=_q8(caproj_w, SW), bcaproj=bcaproj.astype(np.float32),
        wfc=fw.astype(BF), bfc=fb.astype(np.float32),
        wmproj=np.ascontiguousarray(
            mproj_w.reshape(32, P, NCH, P).transpose(2, 1, 0, 3)
        ).reshape(NCH, P, 4 * C).astype(BF),
        bmproj=mproj_b.astype(np.float32),
        wdown=down_w.astype(BF), bdown=down_b.astype(np.float32),
        wup=up_w.astype(BF), bup=up_b.astype(np.float32),
    )

    in_maps = []
    for c in range(8):
        b, hh = c // 2, c % 2
        hs = slice(hh * 512, hh * 512 + 512)
        wqkv = np.concatenate([aw[:, hs], aw[:, C:2 * C][:, hs],
                               aw[:, 2 * C:][:, hs]], 1)
        bqk = np.concatenate([ab[hs], ab[C:2 * C][hs]])
        encp = np.zeros((TEP, C), np.float32)
        encp[:TE] = enc[b]
        xTb = np.ascontiguousarray(x[b].T)
        idx = np.r_[hh * 256:(hh + 1) * 256, 512 + hh * 256:512 + (hh + 1) * 256]
        m = dict(shared)
        m.update(
            xT=xTb.astype(BF),
            x_ownT=np.ascontiguousarray(xTb[:, idx]),
            encT=_q8(np.ascontiguousarray(encp.T), SLN),
            wqkv=_q8(wqkv, SW),
            bqk=bqk.astype(np.float32),
            waproj=_q8(aproj_w[hs], SW),
        )
        in_maps.append(m)
    return in_maps


def kernel(**inputs):
    from concourse.bass_utils import run_bass_kernel_spmd
    if 'nc' not in _BUILT:
        _BUILT['nc'] = _build_nc()
    in_maps = _prep_inputs(inputs)
    res = run_bass_kernel_spmd(_BUILT['nc'], in_maps, core_ids=list(range(8)))
    y = np.zeros((4, T, C), np.float32)
    for c in range(8):
        b, hh = c // 2, c % 2
        idx = np.r_[hh * 256:(hh + 1) * 256, 512 + hh * 256:512 + (hh + 1) * 256]
        y[b, idx, :] = res.results[c]["out"].T
    return y


# revision 37
# speedup vs baseline: 1.1557x; 1.1557x over previous
"""Trainium2 Bass kernel for nn_Block_78022375899354 (dense transformer block).

Sharding (8 cores): core c -> batch b=c//2, head-half hh=c%2.
  Phase 1 (self-attn): head-split -- each core computes q/k/v for its 8 heads
    over the full batch; causal attention produces the core's 512 output
    channels for ALL tokens; a pairwise AllToAll exchanges channel halves so
    each core ends with all 1024 attn channels for its OWN 512 tokens.
  Phase 2+3 (cross-attn, MLP, adapter): token-split (512 tokens per core).

Precision: fp8e4 (DoubleRow, 2x PE) for qkv / aproj / cross-attn q,k,v /
caproj GEMMs with power-of-2 scales folded into drains (verified 3.2e-3
rel err on CPU sim vs 2e-2 budget); bf16 for scores/av/fc/mproj; f32 residual.
All reciprocals via exp(-ln(x)) on ACT so the whole attention region uses one
activation table set (natural_log_exp) -- no table thrash, no slow DVE recip.
Scores for the head pair (2h, 2h+1) run concurrently in PE row-groups 0-63 /
64-127; their exps share one 2-bank ACT op. LN gain/bias and projection
biases folded host-side.
"""
import sys
sys.path.insert(0, '/opt/trn_rl_repo')
import numpy as np
import ml_dtypes

BF = ml_dtypes.bfloat16
F8 = ml_dtypes.float8_e4m3fn
P = 128
C = 1024
T = 1024
TE = 257
TEP = 384          # padded encoder length (3 chunks of 128)
NCH = C // P       # 8 channel chunks
F = 512            # free-dim tile (tokens per core)
H = 16
D = 64
EPS = 1e-5

SW = 1024.0        # fp8 weight scale
SLN = 16.0         # fp8 scale for layernorm outputs + encoder embd
SATT = 32.0        # fp8 scale for attention outputs
DS_QKV = 1.0 / (SW * SLN)      # 2^-14
DS_ATT = 1.0 / (SW * SATT)     # 2^-15
LN32 = float(np.log(SATT))

_BUILT = {}


def _build_nc():
    import concourse.bass as bass
    import concourse.mybir as mybir
    import concourse.tile as tile
    from contextlib import ExitStack

    f32 = mybir.dt.float32
    f32r = mybir.dt.float32r
    bf16 = mybir.dt.bfloat16
    fp8 = mybir.dt.float8e4
    AF = mybir.ActivationFunctionType
    ALU = mybir.AluOpType
    DR = mybir.MatmulPerfMode.DoubleRow

    nc = bass.Bass("TRN2", num_devices=8)

    # ---------------- DRAM I/O ----------------
    xT = nc.dram_tensor("xT", [C, T], bf16, kind="ExternalInput")
    x_ownT = nc.dram_tensor("x_ownT", [C, F], f32, kind="ExternalInput")
    encT = nc.dram_tensor("encT", [C, TEP], fp8, kind="ExternalInput")
    wqkv = nc.dram_tensor("wqkv", [C, 1536], fp8, kind="ExternalInput")
    bqk = nc.dram_tensor("bqk", [1024], f32, kind="ExternalInput")
    waproj = nc.dram_tensor("waproj", [512, C], fp8, kind="ExternalInput")
    battn = nc.dram_tensor("battn", [C], f32, kind="ExternalInput")
    wca = nc.dram_tensor("wca", [C, 3 * C], fp8, kind="ExternalInput")
    bcaqk = nc.dram_tensor("bcaqk", [2 * C], f32, kind="ExternalInput")
    wcaproj = nc.dram_tensor("wcaproj", [C, C], fp8, kind="ExternalInput")
    bcaproj = nc.dram_tensor("bcaproj", [C], f32, kind="ExternalInput")
    wfc = nc.dram_tensor("wfc", [C, 4 * C], bf16, kind="ExternalInput")
    bfc = nc.dram_tensor("bfc", [4 * C], f32, kind="ExternalInput")
    wmproj = nc.dram_tensor("wmproj", [NCH, P, 4 * C], bf16, kind="ExternalInput")
    bmproj = nc.dram_tensor("bmproj", [C], f32, kind="ExternalInput")
    wdown = nc.dram_tensor("wdown", [C, 256], bf16, kind="ExternalInput")
    bdown = nc.dram_tensor("bdown", [256], f32, kind="ExternalInput")
    wup = nc.dram_tensor("wup", [256, C], bf16, kind="ExternalInput")
    bup = nc.dram_tensor("bup", [C], f32, kind="ExternalInput")
    out_d = nc.dram_tensor("out", [C, F], f32, kind="ExternalOutput")

    def r3(ap):
        return ap.rearrange("(o p) f -> p o f", p=P)

    def r2(ap):
        return ap.rearrange("(o p) -> p o", p=P)

    with tile.TileContext(nc) as tc:
        with ExitStack() as ctx:
            consts = ctx.enter_context(tc.tile_pool(name="consts", bufs=1))
            work = ctx.enter_context(tc.tile_pool(name="work", bufs=2))
            lns = ctx.enter_context(tc.tile_pool(name="lns", bufs=2))
            dram = ctx.enter_context(tc.tile_pool(name="dram", bufs=1, space="DRAM"))
            # PSUM budget: 2x [P,2,F] "st" (4 banks) + 2x [P,2,F] "ps1" (4)
            ps2 = ctx.enter_context(tc.tile_pool(name="ps2", bufs=2, space="PSUM"))
            ps1 = ctx.enter_context(tc.tile_pool(name="ps1", bufs=2, space="PSUM"))
            lnxb_pool = ctx.enter_context(tc.tile_pool(name="lnxb_pool", bufs=1))
            x2pool = ctx.enter_context(tc.tile_pool(name="x2pool", bufs=1))
            pool_wfc = ctx.enter_context(tc.tile_pool(name="pool_wfc", bufs=2))

            # ---------- constants ----------
            ones_col_bf = consts.tile([P, 1], bf16)
            nc.vector.memset(ones_col_bf, 1.0)
            ones_f32_tmp = consts.tile([1, P], f32)
            nc.vector.memset(ones_f32_tmp, 1.0)
            ones_row_f32 = consts.tile([1, P], f32r)
            nc.scalar.copy(ones_row_f32, ones_f32_tmp)
            ones_bc = consts.tile([1, D], f32r)
            nc.scalar.copy(ones_bc, ones_f32_tmp[:, 0:D])
            # encoder pad handling: rows>0 of kt-chunk 2 get exp bias -1e30 -> 0
            padbias = consts.tile([P, 1], f32)
            nc.vector.memset(padbias, -1e30)
            nc.vector.memset(padbias[0:1, :], 0.0)
            eps_s = consts.tile([P, 1], f32)        # eps / SLN^2 (ln arg bias)
            nc.vector.memset(eps_s, EPS / (SLN * SLN))
            eps_p = consts.tile([P, 1], f32)
            nc.vector.memset(eps_p, EPS)
            ln32_t = consts.tile([P, 1], f32)
            nc.vector.memset(ln32_t, LN32)

            # ---------- bias tiles ----------
            def bias_tile(dr_t, ncols):
                t = consts.tile([P, ncols], f32, tag=f"b_{dr_t.name}",
                                name=f"b_{dr_t.name}")
                nc.sync.dma_start(t, r2(dr_t[:]))
                return t
            bqk_sb = bias_tile(bqk, 8)
            battn_sb = bias_tile(battn, NCH)
            bcaqk_sb = bias_tile(bcaqk, 16)
            bcaproj_sb = bias_tile(bcaproj, NCH)
            bfc_sb = bias_tile(bfc, 32)
            bmproj_sb = bias_tile(bmproj, NCH)
            bdown_sb = bias_tile(bdown, 2)
            bup_sb = bias_tile(bup, NCH)

            # pools live only through phases 1-2 (freed before the MLP)
            actx = ExitStack()
            exp_pool = actx.enter_context(tc.tile_pool(name="exp_pool", bufs=3))
            anorm = actx.enter_context(tc.tile_pool(name="anorm", bufs=2))
            wstream = actx.enter_context(tc.tile_pool(name="wstream", bufs=2))
            kvpool = actx.enter_context(tc.tile_pool(name="kvpool", bufs=1))

            # ---------- layernorm (feature-major), rsqrt via exp(-.5 ln) ----
            # xb: bf16 [P, NCH, ntok]; ln_out scaled by S (16 for fp8 outs).
            def layernorm(xb, ntok, ln_out, sq_scale, eps_t):
                stats = []
                for nt in range(ntok // F):
                    sl = slice(nt * F, (nt + 1) * F)
                    s1 = ps1.tile([1, F], f32, tag="ps1")
                    s2 = ps1.tile([1, F], f32, tag="ps1")
                    for kc in range(NCH):
                        nc.tensor.matmul(s1, ones_col_bf, xb[:, kc, sl],
                                         start=(kc == 0), stop=(kc == NCH - 1))
                    for kc in range(NCH):
                        xsq = work.tile([P, F], bf16, tag="lnxsq")
                        nc.vector.tensor_mul(xsq, xb[:, kc, sl], xb[:, kc, sl])
                        nc.tensor.matmul(s2, ones_col_bf, xsq,
                                         start=(kc == 0), stop=(kc == NCH - 1))
                    # copy sums out of PSUM promptly to recycle the slots
                    s1r = lns.tile([1, F], f32r, tag="m")
                    nc.scalar.copy(s1r, s1)
                    s2r = lns.tile([1, F], f32r, tag="v")
                    nc.scalar.copy(s2r, s2)
                    stats.append((sl, s1r, s2r))
                for sl, s1r, s2r in stats:
                    # broadcast raw sums to all partitions (f32r single-cycle)
                    psS0 = ps2.tile([P, F], f32, tag="st")
                    psS1 = ps2.tile([P, F], f32, tag="st")
                    nc.tensor.matmul(psS0, ones_row_f32, s1r,
                                     start=True, stop=True)
                    nc.tensor.matmul(psS1, ones_row_f32, s2r,
                                     start=True, stop=True)
                    mt = work.tile([P, F], f32, tag="lnmt")
                    nc.vector.tensor_scalar_mul(mt, psS0, 1.0 / C)
                    var = work.tile([P, F], f32, tag="lnvar")
                    # var = s2/C - m*m
                    nc.vector.scalar_tensor_tensor(
                        var, in0=mt, scalar=-1.0, in1=mt, op0=ALU.mult,
                        op1=ALU.mult)
                    nc.vector.scalar_tensor_tensor(
                        var, in0=psS1, scalar=1.0 / C, in1=var,
                        op0=ALU.mult, op1=ALU.add)
                    # A = S*rsqrt(var+eps) = exp(-0.5 ln(var/S^2 + eps/S^2))
                    nc.scalar.activation(var, var, AF.Ln, bias=eps_t[:, 0:1],
                                         scale=sq_scale)
                    A_sb = work.tile([P, F], bf16, tag="lnA")
                    nc.scalar.activation(A_sb, var, AF.Exp, scale=-0.5)
                    B_sb = work.tile([P, F], bf16, tag="lnB")
                    nc.vector.scalar_tensor_tensor(
                        B_sb, in0=mt, scalar=-1.0, in1=A_sb,
                        op0=ALU.mult, op1=ALU.mult)
                    for kc in range(NCH):
                        tmp = work.tile([P, F], bf16, tag="lntmp")
                        nc.vector.tensor_mul(tmp, xb[:, kc, sl], A_sb)
                        nc.vector.tensor_add(ln_out[:, kc, sl], tmp, B_sb)

            # attention normalize pair: rb = exp(ln32 - ln(den)) = 32/den,
            # broadcast across partitions via PE, multiply numerators (PSUM).
            def attn_norm_pair(pav, dstA, dstB):
                lnden = anorm.tile([1, 2, F], f32r, tag="lnden")
                nc.scalar.activation(lnden, pav[64:65, :, :], AF.Ln)
                pbc = ps2.tile([D, 2, F], f32, tag="st")
                for h in range(2):
                    nc.tensor.matmul(pbc[:, h, :], ones_bc,
                                     lnden[:, h, :],
                                     start=True, stop=True)
                rb = anorm.tile([D, 2, F], bf16, tag="rbc")
                nc.scalar.activation(rb, pbc, AF.Exp, scale=-1.0,
                                     bias=ln32_t[0:D, 0:1])
                nc.vector.tensor_mul(dstA, pav[0:D, 0, :], rb[:, 0, :])
                nc.vector.tensor_mul(dstB, pav[0:D, 1, :], rb[:, 1, :])

            # two half-RSs: shard j of half qt -> rank j's token quarter.
            # partials quantized to fp8 (scale 64) to halve the wire.
            Q = 256
            SRS = 64.0
            cc_in = [dram.tile([2, C, Q], fp8, tag=f"ccin{qt}",
                               name=f"ccin{qt}") for qt in range(2)]
            cc_out = [dram.tile([C, Q], fp8, tag=f"ccout{qt}",
                                name=f"ccout{qt}") for qt in range(2)]

            # =================================================================
            # Phase 1: self-attention (head-split, full batch)
            # =================================================================
            with ExitStack() as p1:
                pool_ln1 = p1.enter_context(tc.tile_pool(name="pool_ln1", bufs=1))
                ln1T = pool_ln1.tile([P, NCH, T], fp8)
                with tc.tile_pool(name="pool_x", bufs=1) as pool_x:
                    xT_sb = pool_x.tile([P, NCH, T], bf16)
                    nc.sync.dma_start(xT_sb, r3(xT[:]))
                    layernorm(xT_sb, T, ln1T, 1.0 / (SLN * SLN), eps_s)

                pool_p1 = p1.enter_context(tc.tile_pool(name="pool_p1", bufs=1))
                # causal diagonal-block mask (duplicated for the head
                # pair): dmask[i, h, j] = 1 iff j >= i, applied to the 128
                # columns at the diagonal; columns left of it are skipped
                # entirely, columns right of it are always live.
                dmask = pool_p1.tile([P, 2, P], bf16)
                for h in range(2):
                    nc.gpsimd.memset(dmask[:, h, :], 1.0)
                    nc.gpsimd.affine_select(
                        out=dmask[:, h, :], in_=dmask[:, h, :],
                        compare_op=ALU.is_ge, fill=0.0,
                        base=0, channel_multiplier=-1, pattern=[[1, P]],
                    )
                pool_wq = ExitStack()
                pool_wq_p = pool_wq.enter_context(
                    tc.tile_pool(name="pool_wq", bufs=1))
                wqkv_sb = pool_wq_p.tile([P, NCH, 1536], fp8)
                nc.sync.dma_start(wqkv_sb, r3(wqkv[:]))

                q_sb = pool_p1.tile([P, 4, T], bf16)
                k_sb = pool_p1.tile([P, 4, T], bf16)
                for m in range(4):
                    for ntk in range(T // F):
                        sl = slice(ntk * F, (ntk + 1) * F)
                        for dst, woff, boff in ((q_sb, 0, 0), (k_sb, 512, 4)):
                            pt = ps1.tile([P, F], f32, tag="ps1")
                            for kcp in range(0, NCH, 2):
                                nc.tensor.matmul(
                                    pt,
                                    wqkv_sb[:, kcp:kcp + 2,
                                            woff + m * P:woff + (m + 1) * P],
                                    ln1T[:, kcp:kcp + 2, sl],
                                    start=(kcp == 0), stop=(kcp == NCH - 2),
                                    perf_mode=DR)
                            nc.scalar.activation(
                                dst[:, m, sl], pt, AF.Identity,
                                bias=bqk_sb[:, boff + m:boff + m + 1],
                                scale=DS_QKV)

                waproj_sb = pool_p1.tile([P, 4, C], fp8)
                nc.sync.dma_start(waproj_sb, r3(waproj[:]))

                v_sb = pool_p1.tile([P, NCH, 8, 65], bf16)
                nc.vector.memset(v_sb[:, :, :, 64:65], 1.0)
                for tkc in range(NCH):
                    pt = ps1.tile([P, F], f32, tag="ps1")
                    for kcp in range(0, NCH, 2):
                        nc.tensor.matmul(
                            pt, ln1T[:, kcp:kcp + 2, tkc * P:(tkc + 1) * P],
                            wqkv_sb[:, kcp:kcp + 2, 1024:1536],
                            start=(kcp == 0), stop=(kcp == NCH - 2),
                            perf_mode=DR)
                    nc.scalar.activation(
                        v_sb[:, tkc, :, 0:64],
                        pt.rearrange("p (h d) -> p h d", h=8),
                        AF.Identity, scale=DS_QKV)

                pool_wq.close()

                # encoder K/V (independent work: fills PE gaps in the
                # LN/attention stretches and overlaps the collectives)
                encT_sb = kvpool.tile([P, NCH, TEP], fp8)
                nc.sync.dma_start(encT_sb, r3(encT[:]))
                kc_sb = kvpool.tile([P, NCH, TEP], bf16)
                wca_k = wstream.tile([P, NCH, C], fp8, tag="w8k")
                nc.sync.dma_start(wca_k, r3(wca[:, C:2 * C]))
                for m in range(NCH):
                    pt = ps1.tile([P, TEP], f32, tag="ps1")
                    for kcp in range(0, NCH, 2):
                        nc.tensor.matmul(pt,
                                         wca_k[:, kcp:kcp + 2, m * P:(m + 1) * P],
                                         encT_sb[:, kcp:kcp + 2, :],
                                         start=(kcp == 0), stop=(kcp == NCH - 2),
                                         perf_mode=DR)
                    nc.vector.tensor_scalar(kc_sb[:, m, :], pt, DS_QKV,
                                            bcaqk_sb[:, 8 + m:8 + m + 1],
                                            op0=ALU.mult, op1=ALU.add)
                vc_sb = kvpool.tile([P, 3, H, 65], bf16)
                nc.vector.memset(vc_sb[:, :, :, 64:65], 1.0)
                wca_v = wstream.tile([P, NCH, C], fp8, tag="w8k")
                nc.sync.dma_start(wca_v, r3(wca[:, 2 * C:3 * C]))
                for tkc in range(3):
                    for nh in range(2):
                        pt = ps1.tile([P, F], f32, tag="ps1")
                        for kcp in range(0, NCH, 2):
                            nc.tensor.matmul(
                                pt, encT_sb[:, kcp:kcp + 2, tkc * P:(tkc + 1) * P],
                                wca_v[:, kcp:kcp + 2, nh * F:(nh + 1) * F],
                                start=(kcp == 0), stop=(kcp == NCH - 2),
                                perf_mode=DR)
                        nc.vector.tensor_scalar_mul(
                            vc_sb[:, tkc, nh * 8:(nh + 1) * 8, 0:64],
                            pt.rearrange("p (h d) -> p h d", h=8), DS_QKV)

                attn_sb = pool_p1.tile([P, 4, T], fp8)
                pending = None
                for qt in range(2):
                    qsl = slice(qt * F, (qt + 1) * F)
                    nkc = 4 * (qt + 1)
                    for hch in range(4):
                        hA, hB = 2 * hch, 2 * hch + 1
                        pav = ps1.tile([65, 2, F], f32, tag="ps1")
                        for kc in range(nkc):
                            ksl = slice(kc * P, (kc + 1) * P)
                            dk = kc - 4 * qt
                            c0 = max(0, P * dk)   # first live column
                            qs2 = slice(qt * F + c0, (qt + 1) * F)
                            st = ps2.tile([P, 2, F], f32, tag="st")
                            nc.tensor.matmul(st[:, 0, c0:],
                                             k_sb[0:64, hch, ksl],
                                             q_sb[0:64, hch, qs2],
                                             start=True, stop=True)
                            nc.tensor.matmul(st[:, 1, c0:],
                                             k_sb[64:128, hch, ksl],
                                             q_sb[64:128, hch, qs2],
                                             start=True, stop=True)
                            e = exp_pool.tile([P, 2, F], bf16, tag="exp")
                            nc.scalar.activation(e[:, :, c0:], st[:, :, c0:],
                                                 AF.Exp, scale=0.125)
                            if dk >= 0:
                                nc.vector.tensor_mul(e[:, :, c0:c0 + P],
                                                     e[:, :, c0:c0 + P],
                                                     dmask)
                            nc.tensor.matmul(pav[:, 0, c0:], v_sb[:, kc, hA, :],
                                             e[:, 0, c0:], start=(kc == 0),
                                             stop=(kc == nkc - 1))
                            nc.tensor.matmul(pav[:, 1, c0:], v_sb[:, kc, hB, :],
                                             e[:, 1, c0:], start=(kc == 0),
                                             stop=(kc == nkc - 1))
                        if pending is not None:
                            attn_norm_pair(*pending)
                        pending = (pav, attn_sb[0:64, hch, qsl],
                                   attn_sb[64:128, hch, qsl])
                    attn_norm_pair(*pending)
                    pending = None
                    # attn-proj partial over my 512 channels for this half,
                    # then reduce-scatter it while later work proceeds
                    part = anorm.tile([P, NCH, F], fp8, tag="part")
                    for m in range(NCH):
                        pt = ps2.tile([P, F], f32, tag="st")
                        for kcp in range(0, 4, 2):
                            nc.tensor.matmul(
                                pt, waproj_sb[:, kcp:kcp + 2, m * P:(m + 1) * P],
                                attn_sb[:, kcp:kcp + 2, qsl],
                                start=(kcp == 0), stop=(kcp == 2),
                                perf_mode=DR)
                        nc.vector.tensor_scalar_mul(part[:, m, :], pt,
                                                    DS_ATT * SRS)
                    for j in range(2):
                        nc.sync.dma_start(
                            r3(cc_in[qt][j]),
                            part[:, :, j * Q:(j + 1) * Q])
                    nc.gpsimd.collective_compute(
                        "ReduceScatter", ALU.add,
                        replica_groups=[[0, 1], [2, 3], [4, 5], [6, 7]],
                        ins=[cc_in[qt][:]], outs=[cc_out[qt][:]])

            # =================================================================
            # Phase 2: cross-attention (token-split, own 512 tokens)
            # =================================================================
            with ExitStack() as p2:
                pool_p2 = p2.enter_context(tc.tile_pool(name="pool_p2", bufs=1))
                # prefetch first MLP weight quarters (overlaps collective+ph2)
                wfcq = {}
                def wfc_quarter(qtr):
                    t = pool_wfc.tile([P, NCH, C], bf16, tag="wfcq",
                                      name=f"wfcq{qtr}")
                    nc.sync.dma_start(t, r3(wfc[:, qtr * C:(qtr + 1) * C]))
                    wfcq[qtr] = t
                wfc_quarter(0)
                wfc_quarter(1)

                # x_own = x + battn + RS'd attn-proj halves
                x_own = pool_p2.tile([P, NCH, F], f32)
                nc.sync.dma_start(x_own, r3(x_ownT[:]))
                rs_sb = pool_p2.tile([P, 2, NCH, Q], fp8)
                for qt in range(2):
                    nc.sync.dma_start(rs_sb[:, qt], r3(cc_out[qt][:]))
                for m in range(NCH):
                    nc.vector.tensor_scalar_add(x_own[:, m, :], x_own[:, m, :],
                                                battn_sb[:, m:m + 1])
                xb2 = lnxb_pool.tile([P, NCH, F], bf16, tag="lnxb")
                for m in range(NCH):
                    for qt in range(2):
                        osl = slice(qt * Q, (qt + 1) * Q)
                        nc.vector.scalar_tensor_tensor(
                            xb2[:, m, osl], in0=rs_sb[:, qt, m, :],
                            scalar=1.0 / SRS,
                            in1=x_own[:, m, osl], op0=ALU.mult, op1=ALU.add)
                for m in range(NCH):
                    for qt in range(2):
                        osl = slice(qt * Q, (qt + 1) * Q)
                        nc.vector.scalar_tensor_tensor(
                            x_own[:, m, osl], in0=rs_sb[:, qt, m, :],
                            scalar=1.0 / SRS,
                            in1=x_own[:, m, osl], op0=ALU.mult, op1=ALU.add)
                ln2T = pool_p2.tile([P, NCH, F], fp8)
                layernorm(xb2, F, ln2T, 1.0 / (SLN * SLN), eps_s)
                # pre-add caproj bias into the residual (consumed at drain)
                for m in range(NCH):
                    nc.vector.tensor_scalar_add(x_own[:, m, :], x_own[:, m, :],
                                                bcaproj_sb[:, m:m + 1])

                qc_sb = pool_p2.tile([P, NCH, F], bf16)
                wca_q = wstream.tile([P, NCH, C], fp8, tag="w8k")
                nc.sync.dma_start(wca_q, r3(wca[:, 0:C]))
                for m in range(NCH):
                    pt = ps1.tile([P, F], f32, tag="ps1")
                    for kcp in range(0, NCH, 2):
                        nc.tensor.matmul(pt,
                                         wca_q[:, kcp:kcp + 2, m * P:(m + 1) * P],
                                         ln2T[:, kcp:kcp + 2, :],
                                         start=(kcp == 0), stop=(kcp == NCH - 2),
                                         perf_mode=DR)
                    nc.vector.tensor_scalar(qc_sb[:, m, :], pt, DS_QKV,
                                            bcaqk_sb[:, m:m + 1],
                                            op0=ALU.mult, op1=ALU.add)

                attnc_sb = pool_p2.tile([P, NCH, F], fp8)
                pending = None
                for hch in range(NCH):
                    hA, hB = 2 * hch, 2 * hch + 1
                    pav = ps1.tile([65, 2, F], f32, tag="ps1")
                    for kc in range(3):
                        ksl = slice(kc * P, (kc + 1) * P)
                        st = ps2.tile([P, 2, F], f32, tag="st")
                        nc.tensor.matmul(st[:, 0, :], kc_sb[0:64, hch, ksl],
                                         qc_sb[0:64, hch, :],
                                         start=True, stop=True)
                        nc.tensor.matmul(st[:, 1, :], kc_sb[64:128, hch, ksl],
                                         qc_sb[64:128, hch, :],
                                         start=True, stop=True)
                        e = exp_pool.tile([P, 2, F], bf16, tag="exp")
                        if kc == 2:
                            nc.scalar.activation(e, st, AF.Exp, scale=0.125,
                                                 bias=padbias[:, 0:1])
                        else:
                            nc.scalar.activation(e, st, AF.Exp, scale=0.125)
                        nc.tensor.matmul(pav[:, 0, :], vc_sb[:, kc, hA, :],
                                         e[:, 0, :], start=(kc == 0),
                                         stop=(kc == 2))
                        nc.tensor.matmul(pav[:, 1, :], vc_sb[:, kc, hB, :],
                                         e[:, 1, :], start=(kc == 0),
                                         stop=(kc == 2))
                    if pending is not None:
                        attn_norm_pair(*pending)
                    pending = (pav, attnc_sb[0:64, hch, :],
                               attnc_sb[64:128, hch, :])
                attn_norm_pair(*pending)

                x2 = x2pool.tile([P, NCH, F], f32)
                wcaproj_sb = wstream.tile([P, NCH, C], fp8, tag="w8k")
                nc.sync.dma_start(wcaproj_sb, r3(wcaproj[:]))
                for m in range(NCH):
                    pt = ps1.tile([P, F], f32, tag="ps1")
                    for kcp in range(0, NCH, 2):
                        nc.tensor.matmul(
                            pt, wcaproj_sb[:, kcp:kcp + 2, m * P:(m + 1) * P],
                            attnc_sb[:, kcp:kcp + 2, :],
                            start=(kcp == 0), stop=(kcp == NCH - 2),
                            perf_mode=DR)
                    nc.vector.scalar_tensor_tensor(
                        x2[:, m, :], in0=pt, scalar=DS_ATT,
                        in1=x_own[:, m, :], op0=ALU.mult, op1=ALU.add)

            actx.close()

            # =================================================================
            # Phase 3: MLP + adapter (token-split)
            # =================================================================
            with ExitStack() as p3:
                pool_p3 = p3.enter_context(tc.tile_pool(name="pool_p3", bufs=1))
                xb3 = lnxb_pool.tile([P, NCH, F], bf16, tag="lnxb")
                for kc in range(NCH):
                    nc.vector.tensor_copy(xb3[:, kc, :], x2[:, kc, :])
                ln3T = pool_p3.tile([P, NCH, F], bf16)
                layernorm(xb3, F, ln3T, 1.0, eps_p)

                gT = pool_p3.tile([P, 32, F], bf16)
                for qtr in range(4):
                    if qtr + 2 < 4:
                        wfc_quarter(qtr + 2)
                    for m8 in range(8):
                        mq = qtr * 8 + m8
                        pt = ps1.tile([P, F], f32, tag="ps1")
                        for kc in range(NCH):
                            nc.tensor.matmul(
                                pt, wfcq[qtr][:, kc, m8 * P:(m8 + 1) * P],
                                ln3T[:, kc, :],
                                start=(kc == 0), stop=(kc == NCH - 1))
                        nc.scalar.activation(gT[:, mq, :], pt,
                                             AF.Gelu_apprx_tanh,
                                             bias=bfc_sb[:, mq:mq + 1])

                h_sb = pool_p3.tile([P, NCH, F], bf16)
                wmp_pool = p3.enter_context(tc.tile_pool(name="wmp_pool", bufs=2))
                for m in range(NCH):
                    wmp_t = wmp_pool.tile([P, 32, P], bf16, tag="wmp")
                    nc.sync.dma_start(
                        wmp_t, wmproj[m].rearrange("p (o f) -> p o f", f=P))
                    pt = ps1.tile([P, F], f32, tag="ps1")
                    for kc in range(32):
                        nc.tensor.matmul(pt, wmp_t[:, kc, :], gT[:, kc, :],
                                         start=(kc == 0), stop=(kc == 31))
                    nc.vector.tensor_scalar_add(h_sb[:, m, :], pt,
                                                bmproj_sb[:, m:m + 1])

                wdown_sb = pool_p3.tile([P, NCH, 256], bf16)
                nc.sync.dma_start(wdown_sb, r3(wdown[:]))
                wup_sb = pool_p3.tile([P, 2, C], bf16)
                nc.sync.dma_start(wup_sb, r3(wup[:]))

                aT = pool_p3.tile([P, 2, F], bf16)
                for m in range(2):
                    pt = ps1.tile([P, F], f32, tag="ps1")
                    for kc in range(NCH):
                        nc.tensor.matmul(pt, wdown_sb[:, kc, m * P:(m + 1) * P],
                                         h_sb[:, kc, :],
                                         start=(kc == 0), stop=(kc == NCH - 1))
                    nc.scalar.activation(aT[:, m, :], pt, AF.Gelu_apprx_tanh,
                                         bias=bdown_sb[:, m:m + 1])

                for m in range(NCH):
                    pt = ps1.tile([P, F], f32, tag="ps1")
                    for kc in range(2):
                        nc.tensor.matmul(pt, wup_sb[:, kc, m * P:(m + 1) * P],
                                         aT[:, kc, :], start=(kc == 0),
                                         stop=(kc == 1))
                    tmp = pool_p3.tile([P, F], f32, tag="fin", bufs=2)
                    nc.vector.scalar_tensor_tensor(
                        tmp, in0=pt, scalar=bup_sb[:, m:m + 1], in1=h_sb[:, m, :],
                        op0=ALU.add, op1=ALU.add)
                    fin = pool_p3.tile([P, F], f32, tag="fin2", bufs=2)
                    nc.vector.tensor_add(fin, tmp, x2[:, m, :])
                    nc.sync.dma_start(out_d[m * P:(m + 1) * P, :], fin)

    _split_sync_waits(nc, mybir)
    return nc


def _split_sync_waits(nc, mybir, maxw=1):
    # walrus rejects instructions with more than a couple of sync waits
    # (e.g. the Tile epilogue Drain waits on every engine + DMA queue);
    # move excess waits onto preceding same-engine no-ops.
    for f in nc.m.functions:
        for bb in f.blocks:
            out, changed = [], False
            for ins in bb.instructions:
                si = ins.sync_info
                if si is not None and len(si.on_wait) > maxw:
                    waits = list(si.on_wait)
                    k = 0
                    while len(waits) > maxw:
                        chunk, waits = waits[:maxw], waits[maxw:]
                        nop = mybir.InstNoOp(name=f"{ins.name}-w{k}", ins=[], outs=[])
                        nop.engine = ins.engine
                        nop.sync_info = mybir.SyncInfo(on_wait=chunk, on_update=[])
                        out.append(nop)
                        k += 1
                    ins.sync_info = mybir.SyncInfo(
                        on_wait=waits, on_update=list(si.on_update))
                    changed = True
                out.append(ins)
            if changed:
                bb.instructions = out


def _q8(a, scale):
    return np.clip(np.asarray(a, np.float32) * scale, -240.0, 240.0).astype(F8)


def _prep_inputs(inputs):
    f = lambda k: np.asarray(inputs[k], np.float32)
    x = f('x')
    enc = f('encoder_embd')
    ln1_g, ln1_b = f('ln1_g'), f('ln1_b')
    ln2_g, ln2_b = f('ln2_g'), f('ln2_b')
    ln3_g, ln3_b = f('ln3_g'), f('ln3_b')
    attn_w, attn_b = f('attn_w'), f('attn_b')
    aproj_w, aproj_b = f('aproj_w'), f('aproj_b')
    ca_w, ca_b = f('ca_w'), f('ca_b')
    caproj_w, caproj_b = f('caproj_w'), f('caproj_b')
    fc_w, fc_b = f('fc_w'), f('fc_b')
    mproj_w, mproj_b = f('mproj_w'), f('mproj_b')
    down_w, down_b = f('down_w'), f('down_b')
    up_w, up_b = f('up_w'), f('up_b')

    # fold LN affine into consuming weights (exact for g=1,b=0 fills)
    aw = ln1_g[:, None] * attn_w
    ab = ln1_b @ attn_w + attn_b
    caw_q = ln2_g[:, None] * ca_w[:, :C]
    cab_q = ln2_b @ ca_w[:, :C] + ca_b[:C]
    fw = ln3_g[:, None] * fc_w
    fb = ln3_b @ fc_w + fc_b

    battn = aproj_b + ab[2 * C:] @ aproj_w            # v-bias folded
    bcaproj = caproj_b + ca_b[2 * C:] @ caproj_w

    wca_full = np.concatenate([caw_q, ca_w[:, C:2 * C], ca_w[:, 2 * C:]], 1)
    bcaqk = np.concatenate([cab_q, ca_b[C:2 * C]]).astype(np.float32)

    shared = dict(
        wca=_q8(wca_full, SW), bcaqk=bcaqk,
        battn=battn.astype(np.float32),
        wcaproj=_q8(caproj_w, SW), bcaproj=bcaproj.astype(np.float32),
        wfc=fw.astype(BF), bfc=fb.astype(np.float32),
        wmproj=np.ascontiguousarray(
            mproj_w.reshape(32, P, NCH, P).transpose(2, 1, 0, 3)
        ).reshape(NCH, P, 4 * C).astype(BF),
        bmproj=mproj_b.astype(np.float32),
        wdown=down_w.astype(BF), bdown=down_b.astype(np.float32),
        wup=up_w.astype(BF), bup=up_b.astype(np.float32),
    )

    in_maps = []
    for c in range(8):
        b, hh = c // 2, c % 2
        hs = slice(hh * 512, hh * 512 + 512)
        wqkv = np.concatenate([aw[:, hs], aw[:, C:2 * C][:, hs],
                               aw[:, 2 * C:][:, hs]], 1)
        bqk = np.concatenate([ab[hs], ab[C:2 * C][hs]])
        encp = np.zeros((TEP, C), np.float32)
        encp[:TE] = enc[b]
        xTb = np.ascontiguousarray(x[b].T)
        idx = np.r_[hh * 256:(hh + 1) * 256, 512 + hh * 256:512 + (hh + 1) * 256]
        m = dict(shared)
        m.update(
            xT=xTb.astype(BF),
            x_ownT=np.ascontiguousarray(xTb[:, idx]),
            encT=_q8(np.ascontiguousarray(encp.T), SLN),
            wqkv=_q8(wqkv, SW),
            bqk=bqk.astype(np.float32),
            waproj=_q8(aproj_w[hs], SW),
        )
        in_maps.append(m)
    return in_maps


def kernel(**inputs):
    from concourse.bass_utils import run_bass_kernel_spmd
    if 'nc' not in _BUILT:
        _BUILT['nc'] = _build_nc()
    in_maps = _prep_inputs(inputs)
    res = run_bass_kernel_spmd(_BUILT['nc'], in_maps, core_ids=list(range(8)))
    y = np.zeros((4, T, C), np.float32)
    for c in range(8):
        b, hh = c // 2, c % 2
        idx = np.r_[hh * 256:(hh + 1) * 256, 512 + hh * 256:512 + (hh + 1) * 256]
        y[b, idx, :] = res.results[c]["out"].T
    return y


# revision 38
# speedup vs baseline: 1.1597x; 1.0035x over previous
"""Trainium2 Bass kernel for nn_Block_78022375899354 (dense transformer block).

Sharding (8 cores): core c -> batch b=c//2, head-half hh=c%2.
  Phase 1 (self-attn): head-split -- each core computes q/k/v for its 8 heads
    over the full batch; causal attention produces the core's 512 output
    channels for ALL tokens; a pairwise AllToAll exchanges channel halves so
    each core ends with all 1024 attn channels for its OWN 512 tokens.
  Phase 2+3 (cross-attn, MLP, adapter): token-split (512 tokens per core).

Precision: fp8e4 (DoubleRow, 2x PE) for qkv / aproj / cross-attn q,k,v /
caproj GEMMs with power-of-2 scales folded into drains (verified 3.2e-3
rel err on CPU sim vs 2e-2 budget); bf16 for scores/av/fc/mproj; f32 residual.
All reciprocals via exp(-ln(x)) on ACT so the whole attention region uses one
activation table set (natural_log_exp) -- no table thrash, no slow DVE recip.
Scores for the head pair (2h, 2h+1) run concurrently in PE row-groups 0-63 /
64-127; their exps share one 2-bank ACT op. LN gain/bias and projection
biases folded host-side.
"""
import sys
sys.path.insert(0, '/opt/trn_rl_repo')
import numpy as np
import ml_dtypes

BF = ml_dtypes.bfloat16
F8 = ml_dtypes.float8_e4m3fn
P = 128
C = 1024
T = 1024
TE = 257
TEP = 384          # padded encoder length (3 chunks of 128)
NCH = C // P       # 8 channel chunks
F = 512            # free-dim tile (tokens per core)
H = 16
D = 64
EPS = 1e-5

SW = 1024.0        # fp8 weight scale
SLN = 16.0         # fp8 scale for layernorm outputs + encoder embd
SATT = 32.0        # fp8 scale for attention outputs
DS_QKV = 1.0 / (SW * SLN)      # 2^-14
DS_ATT = 1.0 / (SW * SATT)     # 2^-15
LN32 = float(np.log(SATT))

_BUILT = {}


def _build_nc():
    import concourse.bass as bass
    import concourse.mybir as mybir
    import concourse.tile as tile
    from contextlib import ExitStack

    f32 = mybir.dt.float32
    f32r = mybir.dt.float32r
    bf16 = mybir.dt.bfloat16
    fp8 = mybir.dt.float8e4
    AF = mybir.ActivationFunctionType
    ALU = mybir.AluOpType
    DR = mybir.MatmulPerfMode.DoubleRow

    nc = bass.Bass("TRN2", num_devices=8)

    # ---------------- DRAM I/O ----------------
    xT = nc.dram_tensor("xT", [C, T], bf16, kind="ExternalInput")
    x_ownT = nc.dram_tensor("x_ownT", [C, F], f32, kind="ExternalInput")
    encT = nc.dram_tensor("encT", [C, TEP], fp8, kind="ExternalInput")
    wqkv = nc.dram_tensor("wqkv", [C, 1536], fp8, kind="ExternalInput")
    bqk = nc.dram_tensor("bqk", [1024], f32, kind="ExternalInput")
    waproj = nc.dram_tensor("waproj", [512, C], fp8, kind="ExternalInput")
    battn = nc.dram_tensor("battn", [C], f32, kind="ExternalInput")
    wca = nc.dram_tensor("wca", [C, 3 * C], fp8, kind="ExternalInput")
    bcaqk = nc.dram_tensor("bcaqk", [2 * C], f32, kind="ExternalInput")
    wcaproj = nc.dram_tensor("wcaproj", [C, C], fp8, kind="ExternalInput")
    bcaproj = nc.dram_tensor("bcaproj", [C], f32, kind="ExternalInput")
    wfc = nc.dram_tensor("wfc", [C, 4 * C], bf16, kind="ExternalInput")
    bfc = nc.dram_tensor("bfc", [4 * C], f32, kind="ExternalInput")
    wmproj = nc.dram_tensor("wmproj", [NCH, P, 4 * C], bf16, kind="ExternalInput")
    bmproj = nc.dram_tensor("bmproj", [C], f32, kind="ExternalInput")
    wdown = nc.dram_tensor("wdown", [C, 256], bf16, kind="ExternalInput")
    bdown = nc.dram_tensor("bdown", [256], f32, kind="ExternalInput")
    wup = nc.dram_tensor("wup", [256, C], bf16, kind="ExternalInput")
    bup = nc.dram_tensor("bup", [C], f32, kind="ExternalInput")
    out_d = nc.dram_tensor("out", [C, F], f32, kind="ExternalOutput")

    def r3(ap):
        return ap.rearrange("(o p) f -> p o f", p=P)

    def r2(ap):
        return ap.rearrange("(o p) -> p o", p=P)

    with tile.TileContext(nc) as tc:
        with ExitStack() as ctx:
            consts = ctx.enter_context(tc.tile_pool(name="consts", bufs=1))
            work = ctx.enter_context(tc.tile_pool(name="work", bufs=2))
            lns = ctx.enter_context(tc.tile_pool(name="lns", bufs=2))
            dram = ctx.enter_context(tc.tile_pool(name="dram", bufs=1, space="DRAM"))
            # PSUM budget: 2x [P,2,F] "st" (4 banks) + 2x [P,2,F] "ps1" (4)
            ps2 = ctx.enter_context(tc.tile_pool(name="ps2", bufs=2, space="PSUM"))
            ps1 = ctx.enter_context(tc.tile_pool(name="ps1", bufs=2, space="PSUM"))
            lnxb_pool = ctx.enter_context(tc.tile_pool(name="lnxb_pool", bufs=1))
            x2pool = ctx.enter_context(tc.tile_pool(name="x2pool", bufs=1))
            pool_wfc = ctx.enter_context(tc.tile_pool(name="pool_wfc", bufs=2))

            # ---------- constants ----------
            ones_col_bf = consts.tile([P, 1], bf16)
            nc.vector.memset(ones_col_bf, 1.0)
            ones_f32_tmp = consts.tile([1, P], f32)
            nc.vector.memset(ones_f32_tmp, 1.0)
            ones_row_f32 = consts.tile([1, P], f32r)
            nc.scalar.copy(ones_row_f32, ones_f32_tmp)
            ones_bc = consts.tile([1, D], f32r)
            nc.scalar.copy(ones_bc, ones_f32_tmp[:, 0:D])
            # encoder pad handling: rows>0 of kt-chunk 2 get exp bias -1e30 -> 0
            padbias = consts.tile([P, 1], f32)
            nc.vector.memset(padbias, -1e30)
            nc.vector.memset(padbias[0:1, :], 0.0)
            eps_s = consts.tile([P, 1], f32)        # eps / SLN^2 (ln arg bias)
            nc.vector.memset(eps_s, EPS / (SLN * SLN))
            eps_p = consts.tile([P, 1], f32)
            nc.vector.memset(eps_p, EPS)
            ln32_t = consts.tile([P, 1], f32)
            nc.vector.memset(ln32_t, LN32)

            # ---------- bias tiles ----------
            def bias_tile(dr_t, ncols):
                t = consts.tile([P, ncols], f32, tag=f"b_{dr_t.name}",
                                name=f"b_{dr_t.name}")
                nc.sync.dma_start(t, r2(dr_t[:]))
                return t
            bqk_sb = bias_tile(bqk, 8)
            battn_sb = bias_tile(battn, NCH)
            bcaqk_sb = bias_tile(bcaqk, 16)
            bcaproj_sb = bias_tile(bcaproj, NCH)
            bfc_sb = bias_tile(bfc, 32)
            bmproj_sb = bias_tile(bmproj, NCH)
            bdown_sb = bias_tile(bdown, 2)
            bup_sb = bias_tile(bup, NCH)

            # pools live only through phases 1-2 (freed before the MLP)
            actx = ExitStack()
            exp_pool = actx.enter_context(tc.tile_pool(name="exp_pool", bufs=3))
            anorm = actx.enter_context(tc.tile_pool(name="anorm", bufs=2))
            wstream = actx.enter_context(tc.tile_pool(name="wstream", bufs=2))
            kvpool = actx.enter_context(tc.tile_pool(name="kvpool", bufs=1))

            # ---------- layernorm (feature-major), rsqrt via exp(-.5 ln) ----
            # xb: bf16 [P, NCH, ntok]; ln_out scaled by S (16 for fp8 outs).
            def layernorm(xb, ntok, ln_out, sq_scale, eps_t):
                stats = []
                for nt in range(ntok // F):
                    sl = slice(nt * F, (nt + 1) * F)
                    s1 = ps1.tile([1, F], f32, tag="ps1")
                    s2 = ps1.tile([1, F], f32, tag="ps1")
                    for kc in range(NCH):
                        nc.tensor.matmul(s1, ones_col_bf, xb[:, kc, sl],
                                         start=(kc == 0), stop=(kc == NCH - 1))
                    for kc in range(NCH):
                        xsq = work.tile([P, F], bf16, tag="lnxsq")
                        nc.vector.tensor_mul(xsq, xb[:, kc, sl], xb[:, kc, sl])
                        nc.tensor.matmul(s2, ones_col_bf, xsq,
                                         start=(kc == 0), stop=(kc == NCH - 1))
                    # copy sums out of PSUM promptly to recycle the slots
                    s1r = lns.tile([1, F], f32r, tag="m")
                    nc.scalar.copy(s1r, s1)
                    s2r = lns.tile([1, F], f32r, tag="v")
                    nc.scalar.copy(s2r, s2)
                    stats.append((sl, s1r, s2r))
                for sl, s1r, s2r in stats:
                    # broadcast raw sums to all partitions (f32r single-cycle)
                    psS0 = ps2.tile([P, F], f32, tag="st")
                    psS1 = ps2.tile([P, F], f32, tag="st")
                    nc.tensor.matmul(psS0, ones_row_f32, s1r,
                                     start=True, stop=True)
                    nc.tensor.matmul(psS1, ones_row_f32, s2r,
                                     start=True, stop=True)
                    mt = work.tile([P, F], f32, tag="lnmt")
                    nc.vector.tensor_scalar_mul(mt, psS0, 1.0 / C)
                    var = work.tile([P, F], f32, tag="lnvar")
                    # var = s2/C - m*m
                    nc.vector.scalar_tensor_tensor(
                        var, in0=mt, scalar=-1.0, in1=mt, op0=ALU.mult,
                        op1=ALU.mult)
                    nc.vector.scalar_tensor_tensor(
                        var, in0=psS1, scalar=1.0 / C, in1=var,
                        op0=ALU.mult, op1=ALU.add)
                    # A = S*rsqrt(var+eps) = exp(-0.5 ln(var/S^2 + eps/S^2))
                    nc.scalar.activation(var, var, AF.Ln, bias=eps_t[:, 0:1],
                                         scale=sq_scale)
                    A_sb = work.tile([P, F], bf16, tag="lnA")
                    nc.scalar.activation(A_sb, var, AF.Exp, scale=-0.5)
                    B_sb = work.tile([P, F], bf16, tag="lnB")
                    nc.vector.scalar_tensor_tensor(
                        B_sb, in0=mt, scalar=-1.0, in1=A_sb,
                        op0=ALU.mult, op1=ALU.mult)
                    for kc in range(NCH):
                        tmp = work.tile([P, F], bf16, tag="lntmp")
                        nc.vector.tensor_mul(tmp, xb[:, kc, sl], A_sb)
                        nc.vector.tensor_add(ln_out[:, kc, sl], tmp, B_sb)

            # attention normalize pair: pav is drained to SBUF right after
            # the last av matmul (freeing its PSUM slot for the next pair);
            # rb = exp(ln32 - ln(den)) = 32/den broadcast across partitions
            # via PE, then bf16 multiplies.
            def pav_drain(pav):
                pv = anorm.tile([65, 2, F], bf16, tag="pavs")
                nc.vector.tensor_copy(pv, pav)
                return pv
            def attn_norm_pair(pv, dstA, dstB):
                lnden = anorm.tile([1, 2, F], f32r, tag="lnden")
                nc.scalar.activation(lnden, pv[64:65, :, :], AF.Ln)
                pbc = ps2.tile([D, 2, F], f32, tag="st")
                for h in range(2):
                    nc.tensor.matmul(pbc[:, h, :], ones_bc,
                                     lnden[:, h, :],
                                     start=True, stop=True)
                rb = anorm.tile([D, 2, F], bf16, tag="rbc")
                nc.scalar.activation(rb, pbc, AF.Exp, scale=-1.0,
                                     bias=ln32_t[0:D, 0:1])
                nc.vector.tensor_mul(dstA, pv[0:D, 0, :], rb[:, 0, :])
                nc.vector.tensor_mul(dstB, pv[0:D, 1, :], rb[:, 1, :])

            # two half-RSs: shard j of half qt -> rank j's token quarter.
            # partials quantized to fp8 (scale 64) to halve the wire.
            Q = 256
            SRS = 64.0
            cc_in = [dram.tile([2, C, Q], fp8, tag=f"ccin{qt}",
                               name=f"ccin{qt}") for qt in range(2)]
            cc_out = [dram.tile([C, Q], fp8, tag=f"ccout{qt}",
                                name=f"ccout{qt}") for qt in range(2)]

            # =================================================================
            # Phase 1: self-attention (head-split, full batch)
            # =================================================================
            with ExitStack() as p1:
                pool_ln1 = p1.enter_context(tc.tile_pool(name="pool_ln1", bufs=1))
                ln1T = pool_ln1.tile([P, NCH, T], fp8)
                with tc.tile_pool(name="pool_x", bufs=1) as pool_x:
                    xT_sb = pool_x.tile([P, NCH, T], bf16)
                    nc.sync.dma_start(xT_sb, r3(xT[:]))
                    layernorm(xT_sb, T, ln1T, 1.0 / (SLN * SLN), eps_s)

                pool_p1 = p1.enter_context(tc.tile_pool(name="pool_p1", bufs=1))
                # causal diagonal-block mask (duplicated for the head
                # pair): dmask[i, h, j] = 1 iff j >= i, applied to the 128
                # columns at the diagonal; columns left of it are skipped
                # entirely, columns right of it are always live.
                dmask = pool_p1.tile([P, 2, P], bf16)
                for h in range(2):
                    nc.gpsimd.memset(dmask[:, h, :], 1.0)
                    nc.gpsimd.affine_select(
                        out=dmask[:, h, :], in_=dmask[:, h, :],
                        compare_op=ALU.is_ge, fill=0.0,
                        base=0, channel_multiplier=-1, pattern=[[1, P]],
                    )
                pool_wq = ExitStack()
                pool_wq_p = pool_wq.enter_context(
                    tc.tile_pool(name="pool_wq", bufs=1))
                wqkv_sb = pool_wq_p.tile([P, NCH, 1536], fp8)
                nc.sync.dma_start(wqkv_sb, r3(wqkv[:]))

                q_sb = pool_p1.tile([P, 4, T], bf16)
                k_sb = pool_p1.tile([P, 4, T], bf16)
                for m in range(4):
                    for ntk in range(T // F):
                        sl = slice(ntk * F, (ntk + 1) * F)
                        for dst, woff, boff in ((q_sb, 0, 0), (k_sb, 512, 4)):
                            pt = ps1.tile([P, F], f32, tag="ps1")
                            for kcp in range(0, NCH, 2):
                                nc.tensor.matmul(
                                    pt,
                                    wqkv_sb[:, kcp:kcp + 2,
                                            woff + m * P:woff + (m + 1) * P],
                                    ln1T[:, kcp:kcp + 2, sl],
                                    start=(kcp == 0), stop=(kcp == NCH - 2),
                                    perf_mode=DR)
                            nc.scalar.activation(
                                dst[:, m, sl], pt, AF.Identity,
                                bias=bqk_sb[:, boff + m:boff + m + 1],
                                scale=DS_QKV)

                waproj_sb = pool_p1.tile([P, 4, C], fp8)
                nc.sync.dma_start(waproj_sb, r3(waproj[:]))

                v_sb = pool_p1.tile([P, NCH, 8, 65], bf16)
                nc.vector.memset(v_sb[:, :, :, 64:65], 1.0)
                for tkc in range(NCH):
                    pt = ps1.tile([P, F], f32, tag="ps1")
                    for kcp in range(0, NCH, 2):
                        nc.tensor.matmul(
                            pt, ln1T[:, kcp:kcp + 2, tkc * P:(tkc + 1) * P],
                            wqkv_sb[:, kcp:kcp + 2, 1024:1536],
                            start=(kcp == 0), stop=(kcp == NCH - 2),
                            perf_mode=DR)
                    nc.scalar.activation(
                        v_sb[:, tkc, :, 0:64],
                        pt.rearrange("p (h d) -> p h d", h=8),
                        AF.Identity, scale=DS_QKV)

                pool_wq.close()

                # encoder K/V (independent work: fills PE gaps in the
                # LN/attention stretches and overlaps the collectives)
                encT_sb = kvpool.tile([P, NCH, TEP], fp8)
                nc.sync.dma_start(encT_sb, r3(encT[:]))
                kc_sb = kvpool.tile([P, NCH, TEP], bf16)
                wca_k = wstream.tile([P, NCH, C], fp8, tag="w8k")
                nc.sync.dma_start(wca_k, r3(wca[:, C:2 * C]))
                for m in range(NCH):
                    pt = ps1.tile([P, TEP], f32, tag="ps1")
                    for kcp in range(0, NCH, 2):
                        nc.tensor.matmul(pt,
                                         wca_k[:, kcp:kcp + 2, m * P:(m + 1) * P],
                                         encT_sb[:, kcp:kcp + 2, :],
                                         start=(kcp == 0), stop=(kcp == NCH - 2),
                                         perf_mode=DR)
                    nc.vector.tensor_scalar(kc_sb[:, m, :], pt, DS_QKV,
                                            bcaqk_sb[:, 8 + m:8 + m + 1],
                                            op0=ALU.mult, op1=ALU.add)
                vc_sb = kvpool.tile([P, 3, H, 65], bf16)
                nc.vector.memset(vc_sb[:, :, :, 64:65], 1.0)
                wca_v = wstream.tile([P, NCH, C], fp8, tag="w8k")
                nc.sync.dma_start(wca_v, r3(wca[:, 2 * C:3 * C]))
                for tkc in range(3):
                    for nh in range(2):
                        pt = ps1.tile([P, F], f32, tag="ps1")
                        for kcp in range(0, NCH, 2):
                            nc.tensor.matmul(
                                pt, encT_sb[:, kcp:kcp + 2, tkc * P:(tkc + 1) * P],
                                wca_v[:, kcp:kcp + 2, nh * F:(nh + 1) * F],
                                start=(kcp == 0), stop=(kcp == NCH - 2),
                                perf_mode=DR)
                        nc.vector.tensor_scalar_mul(
                            vc_sb[:, tkc, nh * 8:(nh + 1) * 8, 0:64],
                            pt.rearrange("p (h d) -> p h d", h=8), DS_QKV)

                attn_sb = pool_p1.tile([P, 4, T], fp8)
                pending = None
                for qt in range(2):
                    qsl = slice(qt * F, (qt + 1) * F)
                    nkc = 4 * (qt + 1)
                    for hch in range(4):
                        hA, hB = 2 * hch, 2 * hch + 1
                        pav = ps1.tile([65, 2, F], f32, tag="ps1")
                        for kc in range(nkc):
                            ksl = slice(kc * P, (kc + 1) * P)
                            dk = kc - 4 * qt
                            c0 = max(0, P * dk)   # first live column
                            qs2 = slice(qt * F + c0, (qt + 1) * F)
                            st = ps2.tile([P, 2, F], f32, tag="st")
                            nc.tensor.matmul(st[:, 0, c0:],
                                             k_sb[0:64, hch, ksl],
                                             q_sb[0:64, hch, qs2],
                                             start=True, stop=True)
                            nc.tensor.matmul(st[:, 1, c0:],
                                             k_sb[64:128, hch, ksl],
                                             q_sb[64:128, hch, qs2],
                                             start=True, stop=True)
                            e = exp_pool.tile([P, 2, F], bf16, tag="exp")
                            nc.scalar.activation(e[:, :, c0:], st[:, :, c0:],
                                                 AF.Exp, scale=0.125)
                            if dk >= 0:
                                nc.vector.tensor_mul(e[:, :, c0:c0 + P],
                                                     e[:, :, c0:c0 + P],
                                                     dmask)
                            nc.tensor.matmul(pav[:, 0, c0:], v_sb[:, kc, hA, :],
                                             e[:, 0, c0:], start=(kc == 0),
                                             stop=(kc == nkc - 1))
                            nc.tensor.matmul(pav[:, 1, c0:], v_sb[:, kc, hB, :],
                                             e[:, 1, c0:], start=(kc == 0),
                                             stop=(kc == nkc - 1))
                        pv = pav_drain(pav)
                        if pending is not None:
                            attn_norm_pair(*pending)
                        pending = (pv, attn_sb[0:64, hch, qsl],
                                   attn_sb[64:128, hch, qsl])
                    attn_norm_pair(*pending)
                    pending = None
                    # attn-proj partial over my 512 channels for this half,
                    # then reduce-scatter it while later work proceeds
                    part = anorm.tile([P, NCH, F], fp8, tag="part")
                    for m in range(NCH):
                        pt = ps2.tile([P, F], f32, tag="st")
                        for kcp in range(0, 4, 2):
                            nc.tensor.matmul(
                                pt, waproj_sb[:, kcp:kcp + 2, m * P:(m + 1) * P],
                                attn_sb[:, kcp:kcp + 2, qsl],
                                start=(kcp == 0), stop=(kcp == 2),
                                perf_mode=DR)
                        if m % 2 == 0:
                            nc.scalar.activation(part[:, m, :], pt,
                                                 AF.Identity,
                                                 scale=DS_ATT * SRS)
                        else:
                            nc.vector.tensor_scalar_mul(part[:, m, :], pt,
                                                        DS_ATT * SRS)
                    for j in range(2):
                        nc.sync.dma_start(
                            r3(cc_in[qt][j]),
                            part[:, :, j * Q:(j + 1) * Q])
                    nc.gpsimd.collective_compute(
                        "ReduceScatter", ALU.add,
                        replica_groups=[[0, 1], [2, 3], [4, 5], [6, 7]],
                        ins=[cc_in[qt][:]], outs=[cc_out[qt][:]])

            # =================================================================
            # Phase 2: cross-attention (token-split, own 512 tokens)
            # =================================================================
            with ExitStack() as p2:
                pool_p2 = p2.enter_context(tc.tile_pool(name="pool_p2", bufs=1))
                # prefetch first MLP weight quarters (overlaps collective+ph2)
                wfcq = {}
                def wfc_quarter(qtr):
                    t = pool_wfc.tile([P, NCH, C], bf16, tag="wfcq",
                                      name=f"wfcq{qtr}")
                    nc.sync.dma_start(t, r3(wfc[:, qtr * C:(qtr + 1) * C]))
                    wfcq[qtr] = t
                wfc_quarter(0)
                wfc_quarter(1)

                # x_own = x + battn + RS'd attn-proj halves
                x_own = pool_p2.tile([P, NCH, F], f32)
                nc.sync.dma_start(x_own, r3(x_ownT[:]))
                rs_sb = pool_p2.tile([P, 2, NCH, Q], fp8)
                for qt in range(2):
                    nc.sync.dma_start(rs_sb[:, qt], r3(cc_out[qt][:]))
                for m in range(NCH):
                    nc.vector.tensor_scalar_add(x_own[:, m, :], x_own[:, m, :],
                                                battn_sb[:, m:m + 1])
                xb2 = lnxb_pool.tile([P, NCH, F], bf16, tag="lnxb")
                for m in range(NCH):
                    for qt in range(2):
                        osl = slice(qt * Q, (qt + 1) * Q)
                        nc.vector.scalar_tensor_tensor(
                            xb2[:, m, osl], in0=rs_sb[:, qt, m, :],
                            scalar=1.0 / SRS,
                            in1=x_own[:, m, osl], op0=ALU.mult, op1=ALU.add)
                for m in range(NCH):
                    for qt in range(2):
                        osl = slice(qt * Q, (qt + 1) * Q)
                        nc.vector.scalar_tensor_tensor(
                            x_own[:, m, osl], in0=rs_sb[:, qt, m, :],
                            scalar=1.0 / SRS,
                            in1=x_own[:, m, osl], op0=ALU.mult, op1=ALU.add)
                ln2T = pool_p2.tile([P, NCH, F], fp8)
                layernorm(xb2, F, ln2T, 1.0 / (SLN * SLN), eps_s)
                # pre-add caproj bias into the residual (consumed at drain)
                for m in range(NCH):
                    nc.vector.tensor_scalar_add(x_own[:, m, :], x_own[:, m, :],
                                                bcaproj_sb[:, m:m + 1])

                qc_sb = pool_p2.tile([P, NCH, F], bf16)
                wca_q = wstream.tile([P, NCH, C], fp8, tag="w8k")
                nc.sync.dma_start(wca_q, r3(wca[:, 0:C]))
                for m in range(NCH):
                    pt = ps1.tile([P, F], f32, tag="ps1")
                    for kcp in range(0, NCH, 2):
                        nc.tensor.matmul(pt,
                                         wca_q[:, kcp:kcp + 2, m * P:(m + 1) * P],
                                         ln2T[:, kcp:kcp + 2, :],
                                         start=(kcp == 0), stop=(kcp == NCH - 2),
                                         perf_mode=DR)
                    nc.vector.tensor_scalar(qc_sb[:, m, :], pt, DS_QKV,
                                            bcaqk_sb[:, m:m + 1],
                                            op0=ALU.mult, op1=ALU.add)

                attnc_sb = pool_p2.tile([P, NCH, F], fp8)
                pending = None
                for hch in range(NCH):
                    hA, hB = 2 * hch, 2 * hch + 1
                    pav = ps1.tile([65, 2, F], f32, tag="ps1")
                    for kc in range(3):
                        ksl = slice(kc * P, (kc + 1) * P)
                        st = ps2.tile([P, 2, F], f32, tag="st")
                        nc.tensor.matmul(st[:, 0, :], kc_sb[0:64, hch, ksl],
                                         qc_sb[0:64, hch, :],
                                         start=True, stop=True)
                        nc.tensor.matmul(st[:, 1, :], kc_sb[64:128, hch, ksl],
                                         qc_sb[64:128, hch, :],
                                         start=True, stop=True)
                        e = exp_pool.tile([P, 2, F], bf16, tag="exp")
                        if kc == 2:
                            nc.scalar.activation(e, st, AF.Exp, scale=0.125,
                                                 bias=padbias[:, 0:1])
                        else:
                            nc.scalar.activation(e, st, AF.Exp, scale=0.125)
                        nc.tensor.matmul(pav[:, 0, :], vc_sb[:, kc, hA, :],
                                         e[:, 0, :], start=(kc == 0),
                                         stop=(kc == 2))
                        nc.tensor.matmul(pav[:, 1, :], vc_sb[:, kc, hB, :],
                                         e[:, 1, :], start=(kc == 0),
                                         stop=(kc == 2))
                    pv = pav_drain(pav)
                    if pending is not None:
                        attn_norm_pair(*pending)
                    pending = (pv, attnc_sb[0:64, hch, :],
                               attnc_sb[64:128, hch, :])
                attn_norm_pair(*pending)

                x2 = x2pool.tile([P, NCH, F], f32)
                wcaproj_sb = wstream.tile([P, NCH, C], fp8, tag="w8k")
                nc.sync.dma_start(wcaproj_sb, r3(wcaproj[:]))
                for m in range(NCH):
                    pt = ps1.tile([P, F], f32, tag="ps1")
                    for kcp in range(0, NCH, 2):
                        nc.tensor.matmul(
                            pt, wcaproj_sb[:, kcp:kcp + 2, m * P:(m + 1) * P],
                            attnc_sb[:, kcp:kcp + 2, :],
                            start=(kcp == 0), stop=(kcp == NCH - 2),
                            perf_mode=DR)
                    nc.vector.scalar_tensor_tensor(
                        x2[:, m, :], in0=pt, scalar=DS_ATT,
                        in1=x_own[:, m, :], op0=ALU.mult, op1=ALU.add)

            actx.close()

            # =================================================================
            # Phase 3: MLP + adapter (token-split)
            # =================================================================
            with ExitStack() as p3:
                pool_p3 = p3.enter_context(tc.tile_pool(name="pool_p3", bufs=1))
                xb3 = lnxb_pool.tile([P, NCH, F], bf16, tag="lnxb")
                for kc in range(NCH):
                    nc.vector.tensor_copy(xb3[:, kc, :], x2[:, kc, :])
                ln3T = pool_p3.tile([P, NCH, F], bf16)
                layernorm(xb3, F, ln3T, 1.0, eps_p)

                gT = pool_p3.tile([P, 32, F], bf16)
                for qtr in range(4):
                    if qtr + 2 < 4:
                        wfc_quarter(qtr + 2)
                    for m8 in range(8):
                        mq = qtr * 8 + m8
                        pt = ps1.tile([P, F], f32, tag="ps1")
                        for kc in range(NCH):
                            nc.tensor.matmul(
                                pt, wfcq[qtr][:, kc, m8 * P:(m8 + 1) * P],
                                ln3T[:, kc, :],
                                start=(kc == 0), stop=(kc == NCH - 1))
                        nc.scalar.activation(gT[:, mq, :], pt,
                                             AF.Gelu_apprx_tanh,
                                             bias=bfc_sb[:, mq:mq + 1])

                h_sb = pool_p3.tile([P, NCH, F], bf16)
                wmp_pool = p3.enter_context(tc.tile_pool(name="wmp_pool", bufs=2))
                for m in range(NCH):
                    wmp_t = wmp_pool.tile([P, 32, P], bf16, tag="wmp")
                    nc.sync.dma_start(
                        wmp_t, wmproj[m].rearrange("p (o f) -> p o f", f=P))
                    pt = ps1.tile([P, F], f32, tag="ps1")
                    for kc in range(32):
                        nc.tensor.matmul(pt, wmp_t[:, kc, :], gT[:, kc, :],
                                         start=(kc == 0), stop=(kc == 31))
                    nc.vector.tensor_scalar_add(h_sb[:, m, :], pt,
                                                bmproj_sb[:, m:m + 1])

                wdown_sb = pool_p3.tile([P, NCH, 256], bf16)
                nc.sync.dma_start(wdown_sb, r3(wdown[:]))
                wup_sb = pool_p3.tile([P, 2, C], bf16)
                nc.sync.dma_start(wup_sb, r3(wup[:]))

                aT = pool_p3.tile([P, 2, F], bf16)
                for m in range(2):
                    pt = ps1.tile([P, F], f32, tag="ps1")
                    for kc in range(NCH):
                        nc.tensor.matmul(pt, wdown_sb[:, kc, m * P:(m + 1) * P],
                                         h_sb[:, kc, :],
                                         start=(kc == 0), stop=(kc == NCH - 1))
                    nc.scalar.activation(aT[:, m, :], pt, AF.Gelu_apprx_tanh,
                                         bias=bdown_sb[:, m:m + 1])

                for m in range(NCH):
                    pt = ps1.tile([P, F], f32, tag="ps1")
                    for kc in range(2):
                        nc.tensor.matmul(pt, wup_sb[:, kc, m * P:(m + 1) * P],
                                         aT[:, kc, :], start=(kc == 0),
                                         stop=(kc == 1))
                    tmp = pool_p3.tile([P, F], f32, tag="fin", bufs=2)
                    nc.vector.scalar_tensor_tensor(
                        tmp, in0=pt, scalar=bup_sb[:, m:m + 1], in1=h_sb[:, m, :],
                        op0=ALU.add, op1=ALU.add)
                    fin = pool_p3.tile([P, F], f32, tag="fin2", bufs=2)
                    nc.vector.tensor_add(fin, tmp, x2[:, m, :])
                    nc.sync.dma_start(out_d[m * P:(m + 1) * P, :], fin)

    _split_sync_waits(nc, mybir)
    return nc


def _split_sync_waits(nc, mybir, maxw=1):
    # walrus rejects instructions with more than a couple of sync waits
    # (e.g. the Tile epilogue Drain waits on every engine + DMA queue);
    # move excess waits onto preceding same-engine no-ops.
    for f in nc.m.functions:
        for bb in f.blocks:
            out, changed = [], False
            for ins in bb.instructions:
                si = ins.sync_info
                if si is not None and len(si.on_wait) > maxw:
                    waits = list(si.on_wait)
                    k = 0
                    while len(waits) > maxw:
                        chunk, waits = waits[:maxw], waits[maxw:]
                        nop = mybir.InstNoOp(name=f"{ins.name}-w{k}", ins=[], outs=[])
                        nop.engine = ins.engine
                        nop.sync_info = mybir.SyncInfo(on_wait=chunk, on_update=[])
                        out.append(nop)
                        k += 1
                    ins.sync_info = mybir.SyncInfo(
                        on_wait=waits, on_update=list(si.on_update))
                    changed = True
                out.append(ins)
            if changed:
                bb.instructions = out


def _q8(a, scale):
    return np.clip(np.asarray(a, np.float32) * scale, -240.0, 240.0).astype(F8)


def _prep_inputs(inputs):
    f = lambda k: np.asarray(inputs[k], np.float32)
    x = f('x')
    enc = f('encoder_embd')
    ln1_g, ln1_b = f('ln1_g'), f('ln1_b')
    ln2_g, ln2_b = f('ln2_g'), f('ln2_b')
    ln3_g, ln3_b = f('ln3_g'), f('ln3_b')
    attn_w, attn_b = f('attn_w'), f('attn_b')
    aproj_w, aproj_b = f('aproj_w'), f('aproj_b')
    ca_w, ca_b = f('ca_w'), f('ca_b')
    caproj_w, caproj_b = f('caproj_w'), f('caproj_b')
    fc_w, fc_b = f('fc_w'), f('fc_b')
    mproj_w, mproj_b = f('mproj_w'), f('mproj_b')
    down_w, down_b = f('down_w'), f('down_b')
    up_w, up_b = f('up_w'), f('up_b')

    # fold LN affine into consuming weights (exact for g=1,b=0 fills)
    aw = ln1_g[:, None] * attn_w
    ab = ln1_b @ attn_w + attn_b
    caw_q = ln2_g[:, None] * ca_w[:, :C]
    cab_q = ln2_b @ ca_w[:, :C] + ca_b[:C]
    fw = ln3_g[:, None] * fc_w
    fb = ln3_b @ fc_w + fc_b

    battn = aproj_b + ab[2 * C:] @ aproj_w            # v-bias folded
    bcaproj = caproj_b + ca_b[2 * C:] @ caproj_w

    wca_full = np.concatenate([caw_q, ca_w[:, C:2 * C], ca_w[:, 2 * C:]], 1)
    bcaqk = np.concatenate([cab_q, ca_b[C:2 * C]]).astype(np.float32)

    shared = dict(
        wca=_q8(wca_full, SW), bcaqk=bcaqk,
        battn=battn.astype(np.float32),
        wcaproj=_q8(caproj_w, SW), bcaproj=bcaproj.astype(np.float32),
        wfc=fw.astype(BF), bfc=fb.astype(np.float32),
        wmproj=np.ascontiguousarray(
            mproj_w.reshape(32, P, NCH, P).transpose(2, 1, 0, 3)
        ).reshape(NCH, P, 4 * C).astype(BF),
        bmproj=mproj_b.astype(np.float32),
        wdown=down_w.astype(BF), bdown=down_b.astype(np.float32),
        wup=up_w.astype(BF), bup=up_b.astype(np.float32),
    )

    in_maps = []
    for c in range(8):
        b, hh = c // 2, c % 2
        hs = slice(hh * 512, hh * 512 + 512)
        wqkv = np.concatenate([aw[:, hs], aw[:, C:2 * C][:, hs],
                               aw[:, 2 * C:][:, hs]], 1)
        bqk = np.concatenate([ab[hs], ab[C:2 * C][hs]])
        encp = np.zeros((TEP, C), np.float32)
        encp[:TE] = enc[b]
        xTb = np.ascontiguousarray(x[b].T)
        idx = np.r_[hh * 256:(hh + 1) * 256, 512 + hh * 256:512 + (hh + 1) * 256]
        m = dict(shared)
        m.update(
            xT=xTb.astype(BF),
            x_ownT=np.ascontiguousarray(xTb[:, idx]),
            encT=_q8(np.ascontiguousarray(encp.T), SLN),
            wqkv=_q8(wqkv, SW),
            bqk=bqk.astype(np.float32),
            waproj=_q8(aproj_w[hs], SW),
        )
        in_maps.append(m)
    return in_maps


def kernel(**inputs):
    from concourse.bass_utils import run_bass_kernel_spmd
    if 'nc' not in _BUILT:
        _BUILT['nc'] = _build_nc()
    in_maps = _prep_inputs(inputs)
    res = run_bass_kernel_spmd(_BUILT['nc'], in_maps, core_ids=list(range(8)))
    y = np.zeros((4, T, C), np.float32)
    for c in range(8):
        b, hh = c // 2, c % 2
        idx = np.r_[hh * 256:(hh + 1) * 256, 512 + hh * 256:512 + (hh + 1) * 256]
        y[b, idx, :] = res.results[c]["out"].T
    return y
